# revision 22
# baseline (speedup 1.0000x reference)
"""CausalTemporalAttnBlock Trainium2 kernel.

Problem: out = x + Wp @ attn(norm(x)) + bp, where norm is GroupNorm(1 group)
over (c,t,h,w) per batch, attention is causal over t, independent per (b,h,w).
Shapes: x (2, 512, 64, 32, 32) fp32; four (512,512) weights + biases.

Strategy (8 NeuronCores, zero communication except a 8-byte AllReduce for
the GroupNorm stats):
  - core i handles batch i//4, h-rows [8*(i%4), 8*(i%4)+8), all w: 256 (h,w)
    locations per core.
  - Whole matmul datapath in bf16 (fp32 PSUM accumulation): full-rate PE
    streaming (fp32 is half rate) and fast weight load. Host quantizes x
    and the folded weights to bf16 (RNE); error stays ~1e-3 vs the 2e-2
    budget.
  - Host folds gamma/beta into the projection weights:
        q = r*(Aq @ x) + (cq - mu*r*uq),  Aq = wq*diag(gamma) (q pre-scaled
    by 1/sqrt(c)), uq = wq@gamma, cq = bq + wq@beta; same for k. The V-path
    affine is folded all the way into the P-projection eviction:
        out = x + r*(Wp @ attn @ (Av x)^T) + (Wp @ dv + bp), dv = cv - mu*r*uv
    (softmax rows sum to 1, so the V bias passes through attention as a
    per-channel constant). mu, r=rstd computed on device (AllReduce of
    per-batch sum/sumsq across the 4 cores of each batch).
  - Host re-lays the shard w-major: [8 h-rows][512 c][32 w * 64 t], so one
    attention group (8 w-locations) is a contiguous 512-column slice.
  - Locations are processed in PAIRS sharing the 128-wide stationary
    operand: VT for 2 locations in one matmul (x-pair stationary, Wv
    moving), scores S^T = K^T Q per pair as one [128,128] matmul (cross
    terms masked off with the causal mask), and AV as a full-K [128,128]
    matmul per pair (masked zeros in attn kill the cross contributions).
  - No max-subtraction in softmax (scores are O(1)); causal+pair mask is a
    0/1 multiply after exp; normalization by 1/rowsum via ones-matmul
    reductions/broadcasts on the PE.
"""

import numpy as np
import ml_dtypes

import concourse.bass as bass
import concourse.tile as tile
from concourse import bacc, mybir
from concourse.bass_utils import run_bass_kernel_spmd

P = 128
B, C, T, H, W = 2, 512, 64, 32, 32
NCORES = 8
HSH = H // 4          # 8 h-rows per core
CCH = C // P          # 4 c chunks
GRP = 8               # locations per attention group
NGRP = W // GRP       # 4 groups per h-row block
NPR = GRP // 2        # 4 location-pairs per group
WT = W * T            # 2048 free columns per (h-row, c) plane
EPS = 1e-6

f32 = mybir.dt.float32
bf16 = mybir.dt.bfloat16
AX = mybir.AxisListType.X
ALU = mybir.AluOpType
AF = mybir.ActivationFunctionType
BF = ml_dtypes.bfloat16


def build_nc(num_cores=NCORES, nblk=HSH, norm_n=None, replica_groups=None,
             use_collective=True):
    if norm_n is None:
        norm_n = C * T * H * W
    if replica_groups is None:
        replica_groups = [[0, 1, 2, 3], [4, 5, 6, 7]]
    nc = bacc.Bacc("TRN2", target_bir_lowering=False, debug=False,
                   num_devices=num_cores)

    xs = nc.declare_dram_parameter("xs", [nblk, C, WT], bf16, isOutput=False)
    wts = {}
    for nm in ("q", "k", "v", "p"):
        wts[nm] = nc.declare_dram_parameter(f"w{nm}t", [C, C], bf16,
                                            isOutput=False)
    ucol = nc.declare_dram_parameter("ucol", [P, 3 * CCH], f32, isOutput=False)
    ccol = nc.declare_dram_parameter("ccol", [P, 3 * CCH], f32, isOutput=False)
    bpcol = nc.declare_dram_parameter("bpcol", [P, CCH], f32, isOutput=False)
    maskp = nc.declare_dram_parameter("maskt", [P, NPR * P], bf16,
                                      isOutput=False)
    ones_row_f = nc.declare_dram_parameter("ones_row_f", [1, P], f32,
                                           isOutput=False)
    ones_col_b = nc.declare_dram_parameter("ones_col_b", [P, 1], bf16,
                                           isOutput=False)
    ones_mat_b = nc.declare_dram_parameter("ones_mat_b", [P, P], bf16,
                                           isOutput=False)
    outp = nc.declare_dram_parameter("out", [nblk, C, WT], f32, isOutput=True)
    cc_in = nc.dram_tensor("cc_in", [1, 2], f32)
    cc_out = nc.dram_tensor("cc_out", [1, 2], f32)

    with tile.TileContext(nc) as tc:
        with (
            tc.tile_pool(name="const", bufs=1) as const,
            tc.tile_pool(name="scal", bufs=1) as sc,
            tc.tile_pool(name="statp", bufs=4) as statp,
            tc.tile_pool(name="sqp", bufs=2) as sqp,
            tc.tile_pool(name="xpool", bufs=2) as xpool,
            tc.tile_pool(name="gpool", bufs=8) as gpool,
            tc.tile_pool(name="spool", bufs=2) as spool,
            tc.tile_pool(name="opool", bufs=4) as opool,
            tc.tile_pool(name="pp", bufs=3, space="PSUM") as pp,
            tc.tile_pool(name="pss", bufs=2, space="PSUM") as pss,
            tc.tile_pool(name="scp", bufs=2, space="PSUM") as scp,
            tc.tile_pool(name="psm", bufs=1, space="PSUM") as psm,
        ):
            # ---------- constants ----------
            # weights as separate contiguous [128,128] tiles (one per
            # stationary) — lets the weight loader run at full rate
            w_sb = {}
            wt_sb = {}
            for nm in ("q", "k", "v", "p"):
                for ci in range(CCH):
                    t = const.tile([P, C], bf16, tag=f"w{nm}{ci}")
                    nc.sync.dma_start(t[:], wts[nm][ci * P:(ci + 1) * P, :])
                    w_sb[nm, ci] = t
                    for co in range(CCH):
                        tt = const.tile([P, P], bf16, tag=f"w{nm}{ci}_{co}")
                        nc.sync.dma_start(
                            tt[:], wts[nm][ci * P:(ci + 1) * P,
                                           co * P:(co + 1) * P])
                        wt_sb[nm, ci, co] = tt
            ucol_sb = const.tile([P, 3 * CCH], f32, tag="ucol")
            nc.sync.dma_start(ucol_sb[:], ucol[:])
            ccol_sb = const.tile([P, 3 * CCH], f32, tag="ccol")
            nc.sync.dma_start(ccol_sb[:], ccol[:])
            bpcol_sb = const.tile([P, CCH], f32, tag="bpcol")
            nc.sync.dma_start(bpcol_sb[:], bpcol[:])
            mask_sb = const.tile([P, NPR * P], bf16, tag="maskt")
            nc.sync.dma_start(mask_sb[:], maskp[:])
            ocb_sb = const.tile([P, 1], bf16, tag="ocb")
            nc.sync.dma_start(ocb_sb[:], ones_col_b[:])
            orf_sb = const.tile([1, P], f32, tag="orf")
            nc.sync.dma_start(orf_sb[:], ones_row_f[:])
            omb_sb = const.tile([P, P], bf16, tag="omb")
            nc.sync.dma_start(omb_sb[:], ones_mat_b[:])

            # ---------- stats (sum / sumsq over the whole shard) ----------
            # x-sum via ones-matmuls accumulating in one PSUM bank (PE is
            # idle during the stats phase); sumsq fused into the Square
            # activation's accum_out — DVE does almost no stats work
            ssq = sc.tile([P, nblk * CCH], f32, tag="ssq")
            ps_sum1 = psm.tile([1, 512], f32, tag="psm")
            nt = nblk * CCH
            for blk in range(nblk):
                for ci in range(CCH):
                    xt = statp.tile([P, WT], bf16, tag="xstat")
                    nc.sync.dma_start(xt[:], xs[blk, ci * P:(ci + 1) * P, :])
                    i = blk * CCH + ci
                    for j in range(WT // 512):
                        nc.tensor.matmul(
                            ps_sum1[:], ocb_sb[:],
                            xt[:, j * 512:(j + 1) * 512],
                            start=(i == 0 and j == 0),
                            stop=(i == nt - 1 and j == WT // 512 - 1),
                            skip_group_check=True)
                    sq = sqp.tile([P, WT], bf16, tag="sq")
                    if i % 3 == 2:
                        # spread the square+reduce work across DVE too so the
                        # stats phase isn't paced by ScalarE alone
                        nc.vector.tensor_mul(sq[:], xt[:], xt[:])
                        nc.vector.reduce_sum(out=ssq[:, i:i + 1], in_=sq[:],
                                             axis=AX)
                    else:
                        nc.scalar.activation(sq[:], xt[:], AF.Square,
                                             accum_out=ssq[:, i:i + 1])
            st_sb = sc.tile([1, 2], f32, tag="st_sb")
            nc.vector.reduce_sum(out=st_sb[0:1, 0:1], in_=ps_sum1[:], axis=AX)
            nc.gpsimd.reduce_sum(out=st_sb[0:1, 1:2], in_=ssq[:],
                                 axis=mybir.AxisListType.XYZWC)
            nc.gpsimd.dma_start(cc_in[:], st_sb[:])
            if use_collective:
                nc.gpsimd.collective_compute(
                    "AllReduce", ALU.add, replica_groups=replica_groups,
                    ins=[cc_in[:]], outs=[cc_out[:]])
            else:
                nc.gpsimd.dma_start(cc_out[:], cc_in[:])
            stg = sc.tile([1, 2], f32, tag="stg")
            nc.gpsimd.dma_start(stg[:], cc_out[:])

            mean = sc.tile([1, 1], f32, tag="mean")
            nc.scalar.activation(mean[:], stg[:, 0:1], AF.Copy,
                                 bias=0.0, scale=1.0 / norm_n)
            ex2 = sc.tile([1, 1], f32, tag="ex2")
            nc.scalar.activation(ex2[:], stg[:, 1:2], AF.Copy,
                                 bias=0.0, scale=1.0 / norm_n)
            msq = sc.tile([1, 1], f32, tag="msq")
            nc.scalar.activation(msq[:], mean[:], AF.Square)
            varp = sc.tile([1, 1], f32, tag="varp")
            nc.vector.tensor_scalar(varp[:], ex2[:], msq[:], EPS,
                                    ALU.subtract, ALU.add)
            sqv = sc.tile([1, 1], f32, tag="sqv")      # = 1/rstd
            nc.scalar.activation(sqv[:], varp[:], AF.Sqrt)
            rst = sc.tile([1, 1], f32, tag="rst")      # = rstd
            nc.vector.reciprocal(rst[:], sqv[:])
            rmu = sc.tile([1, 1], f32, tag="rmu")      # = rstd*mean
            nc.vector.tensor_scalar(rmu[:], mean[:], rst[:], None, ALU.mult)
            vals = sc.tile([1, 3], f32, tag="vals")
            nc.vector.tensor_copy(vals[:, 0:1], rst[:])
            nc.vector.tensor_copy(vals[:, 1:2], rmu[:])
            nc.vector.tensor_copy(vals[:, 2:3], sqv[:])
            # broadcast (rstd, rstd*mean, 1/rstd) across partitions (K=1 mm)
            ps_b = psm.tile([P, 512], f32, tag="psm")
            nc.tensor.matmul(ps_b[:, 0:3], orf_sb[:], vals[:],
                             start=True, stop=True)
            rb = sc.tile([P, 3], f32, tag="rb")
            nc.vector.tensor_copy(rb[:], ps_b[:, 0:3])
            # all-(1/r) stationary for the softmax denominator matmul: the
            # rowsum matmul then directly yields Z/r, whose reciprocal is the
            # r/Z factor applied at the AV eviction
            oiv = sc.tile([P, P], bf16, tag="oiv")
            nc.vector.tensor_scalar(oiv[:], omb_sb[:], rb[:, 2:3], None,
                                    ALU.mult)
            # per-(proj,chunk) eviction biases for q,k,v: D = ccol - rmu*ucol
            dcol = sc.tile([P, 3 * CCH], f32, tag="dcol")
            nc.vector.tensor_scalar(dcol[:], ucol_sb[:], rb[:, 1:2], None,
                                    ALU.mult)
            nc.vector.tensor_sub(dcol[:], ccol_sb[:], dcol[:])
            # dp = Wp @ dv + bp  (per-channel constant added at P-eviction)
            dvb = sc.tile([P, CCH], bf16, tag="dvb")
            nc.vector.tensor_copy(dvb[:], dcol[:, 2 * CCH:3 * CCH])
            ps_dp = psm.tile([P, 512], f32, tag="psm")
            for co in range(CCH):
                for ci in range(CCH):
                    nc.tensor.matmul(
                        ps_dp[:, co:co + 1],
                        w_sb["p", ci][:, co * P:(co + 1) * P],
                        dvb[:, ci:ci + 1], start=(co == 0 and ci == 0),
                        stop=(ci == CCH - 1), skip_group_check=True)
            dp = sc.tile([P, CCH], f32, tag="dp")
            nc.vector.tensor_add(dp[:], ps_dp[:, 0:CCH], bpcol_sb[:])

            # ---------- main blocks ----------
            # One-group software pipeline: stage 1 of group g (projections,
            # scores, exp+mask) is emitted before stage 2 of group g-1
            # (rowsum, AV, P, out), so the softmax latency of g hides under
            # the projection matmuls of g and AV/P of g-1 — the in-order PE
            # queue never waits on ScalarE/DVE.

            def stage1(xb, cs):
                # Q, K projections: psum[co, (w,t)] over ci
                qk = {}
                for pi, nm in enumerate(("q", "k")):
                    for co in range(CCH):
                        ps = pp.tile([P, 512], f32, tag="pp")
                        for ci in range(CCH):
                            nc.tensor.matmul(
                                ps[:], wt_sb[nm, ci, co][:],
                                xb[ci][:, cs:cs + 512], start=(ci == 0),
                                stop=(ci == CCH - 1))
                        t = gpool.tile([P, 512], bf16, tag=f"{nm}g")
                        d = pi * CCH + co
                        # affine eviction on ScalarE (closer to PSUM, keeps
                        # DVE free): t = ps*r + dcol
                        nc.scalar.activation(
                            t[:], ps[:], AF.Identity,
                            bias=dcol[:, d:d + 1], scale=rb[:, 0:1])
                        qk[nm, co] = t

                # VT (raw): per loc PAIR, [128 (2w,s), 512 co]
                vtp = []
                for p in range(NPR):
                    ps = pss.tile([P, 512], f32, tag="ppv")
                    for ci in range(CCH):
                        nc.tensor.matmul(
                            ps[:], xb[ci][:, cs + p * P:cs + (p + 1) * P],
                            w_sb["v", ci][:], start=(ci == 0),
                            stop=(ci == CCH - 1))
                    t = gpool.tile([P, 512], bf16, tag="vtg")
                    nc.scalar.copy(t[:], ps[:])
                    vtp.append(t)

                # scores S^T[(2w,s), (2w,t)] per pair; 4 pair-chains share
                # one PSUM bank: the very first matmul start=True zeroes the
                # bank, later chains' first matmuls overwrite (has_written
                # cleared) and accumulate over ci.
                ps_s = scp.tile([P, 512], f32, tag="pss")
                for p in range(NPR):
                    for ci in range(CCH):
                        nc.tensor.matmul(
                            ps_s[:, p * P:(p + 1) * P],
                            qk["k", ci][:, p * P:(p + 1) * P],
                            qk["q", ci][:, p * P:(p + 1) * P],
                            start=(p == 0 and ci == 0),
                            stop=(ci == CCH - 1), skip_group_check=True)
                # unnormalized masked softmax numerator (normalization is
                # folded into the AV eviction as r/Z)
                pexp = spool.tile([P, 512], bf16, tag="pexp")
                nc.scalar.activation(pexp[:], ps_s[:], AF.Exp)
                pm = spool.tile([P, 512], bf16, tag="pmask")
                nc.vector.tensor_mul(pm[:], pexp[:], mask_sb[:])
                return vtp, pm

            def stage2(st):
                xb, cs, blk, vtp, pm = st
                # rowsum matmul with all-(1/r) stationary => Z/r, broadcast
                # across partitions; fast-approx reciprocal gives r/Z with
                # ~18 good bits, plenty for the bf16 og tiles
                ps_sum = psm.tile([P, 512], f32, tag="psm")
                nc.tensor.matmul(ps_sum[:], oiv[:], pm[:],
                                 start=True, stop=True)
                rz = spool.tile([P, 512], f32, tag="rz")
                nc.vector.reciprocal_approx_fast(out=rz[:], in_=ps_sum[:])

                # AV: O[c,(2w,t)] per pair, full-K (mask zeros kill the
                # cross-location contributions); eviction applies r/Z
                og = []
                for ch in range(CCH):
                    ps_o = pp.tile([P, 512], f32, tag="pp")
                    for p in range(NPR):
                        nc.tensor.matmul(
                            ps_o[:, p * P:(p + 1) * P],
                            vtp[p][:, ch * P:(ch + 1) * P],
                            pm[:, p * P:(p + 1) * P],
                            start=(p == 0), stop=True,
                            skip_group_check=True)
                    t = gpool.tile([P, 512], bf16, tag="og")
                    nc.vector.tensor_mul(t[:], ps_o[:], rz[:])
                    og.append(t)

                # P-projection + bias + residual
                for co in range(CCH):
                    ps = pp.tile([P, 512], f32, tag="pp")
                    for ci in range(CCH):
                        nc.tensor.matmul(
                            ps[:], wt_sb["p", ci, co][:],
                            og[ci][:], start=(ci == 0),
                            stop=(ci == CCH - 1))
                    slab = opool.tile([P, 512], f32, tag="oslab")
                    nc.vector.tensor_scalar(
                        slab[:], ps[:], dp[:, co:co + 1], None, ALU.add)
                    nc.vector.tensor_add(slab[:], slab[:],
                                         xb[co][:, cs:cs + 512])
                    nc.sync.dma_start(
                        outp[blk, co * P:(co + 1) * P, cs:cs + 512],
                        slab[:])

            pending = None
            for blk in range(nblk):
                xb = []
                for ci in range(CCH):
                    t = xpool.tile([P, WT], bf16, tag=f"xb{ci}")
                    nc.sync.dma_start(t[:], xs[blk, ci * P:(ci + 1) * P, :])
                    xb.append(t)
                for g in range(NGRP):
                    cs = g * GRP * T          # 512-col slice of this group
                    vtp, pm = stage1(xb, cs)
                    if pending is not None:
                        stage2(pending)
                    pending = (xb, cs, blk, vtp, pm)
            stage2(pending)
    nc.compile()
    return nc


def host_prep(gamma, beta, wq, bq, wk, bk, wv, bv, wp, bp):
    """Fold gamma/beta into weights; build all constant tensors."""
    s = 1.0 / np.sqrt(np.float32(C))
    g = gamma.astype(np.float64)

    def fold(w, bias, scale):
        a = (w.astype(np.float64) * g[None, :]) * scale      # (co, ci)
        u = (w.astype(np.float64) @ g) * scale               # (co,)
        c0 = (bias.astype(np.float64) + w.astype(np.float64) @
              beta.astype(np.float64)) * scale
        return (np.ascontiguousarray(a.T).astype(BF),
                u.astype(np.float32), c0.astype(np.float32))

    aqt, uq, cq = fold(wq, bq, s)
    akt, uk, ck = fold(wk, bk, 1.0)
    avt, uv, cv = fold(wv, bv, 1.0)
    apt = np.ascontiguousarray(wp.T.astype(np.float32)).astype(BF)

    ucol = np.empty((P, 3 * CCH), np.float32)
    ccol = np.empty((P, 3 * CCH), np.float32)
    for pi, (u, c0) in enumerate(((uq, cq), (uk, ck), (uv, cv))):
        for ch in range(CCH):
            ucol[:, pi * CCH + ch] = u[ch * P:(ch + 1) * P]
            ccol[:, pi * CCH + ch] = c0[ch * P:(ch + 1) * P]
    bpcol = np.empty((P, CCH), np.float32)
    for ch in range(CCH):
        bpcol[:, ch] = bp[ch * P:(ch + 1) * P]

    # pair mask [128, 4*128]: diag 64x64 halves get causal triu (s<=t),
    # off-diag (cross-location) halves are zero; identical per pair.
    tri = np.triu(np.ones((T, T), np.float32))
    blkm = np.zeros((P, P), np.float32)
    blkm[0:T, 0:T] = tri
    blkm[T:2 * T, T:2 * T] = tri
    maskt = np.tile(blkm, (1, NPR))

    consts = {
        "wqt": aqt, "wkt": akt, "wvt": avt, "wpt": apt,
        "ucol": ucol, "ccol": ccol, "bpcol": bpcol,
        "maskt": maskt.astype(BF),
        "ones_row_f": np.ones((1, P), np.float32),
        "ones_col_b": np.ones((P, 1), BF),
        "ones_mat_b": np.ones((P, P), BF),
    }
    return consts


_NC_CACHE = {}


def kernel(x, gamma, beta, wq, bq, wk, bk, wv, bv, wp, bp):
    x = np.asarray(x, np.float32)
    args = [np.asarray(a, np.float32) for a in
            (gamma, beta, wq, bq, wk, bk, wv, bv, wp, bp)]
    consts = host_prep(*args)

    if "nc" not in _NC_CACHE:
        _NC_CACHE["nc"] = build_nc()
    nc = _NC_CACHE["nc"]

    in_maps = []
    for core in range(NCORES):
        b, hg = core // 4, core % 4
        shard = x[b, :, :, hg * HSH:(hg + 1) * HSH, :]        # (C,T,HSH,W)
        shard = np.ascontiguousarray(
            shard.transpose(2, 0, 3, 1)).reshape(HSH, C, WT)  # w-major
        in_maps.append({"xs": shard.astype(BF), **consts})

    global _last_in_maps
    _last_in_maps = in_maps
    res = run_bass_kernel_spmd(nc, in_maps, list(range(NCORES)))

    out = np.empty((B, C, T, H, W), np.float32)
    for core in range(NCORES):
        b, hg = core // 4, core % 4
        o = res.results[core]["out"].reshape(HSH, C, W, T)
        out[b, :, :, hg * HSH:(hg + 1) * HSH, :] = o.transpose(1, 3, 0, 2)
    return out


# revision 26
# speedup vs baseline: 1.0916x; 1.0916x over previous
"""CausalTemporalAttnBlock Trainium2 kernel.

Problem: out = x + Wp @ attn(norm(x)) + bp, where norm is GroupNorm(1 group)
over (c,t,h,w) per batch, attention is causal over t, independent per (b,h,w).
Shapes: x (2, 512, 64, 32, 32) fp32; four (512,512) weights + biases.

Strategy (8 NeuronCores, zero communication except a 8-byte AllReduce for
the GroupNorm stats):
  - core i handles batch i//4, h-rows [8*(i%4), 8*(i%4)+8), all w: 256 (h,w)
    locations per core.
  - Whole matmul datapath in bf16 (fp32 PSUM accumulation): full-rate PE
    streaming (fp32 is half rate) and fast weight load. Host quantizes x
    and the folded weights to bf16 (RNE); error stays ~1e-3 vs the 2e-2
    budget.
  - Host folds gamma/beta into the projection weights:
        q = r*(Aq @ x) + (cq - mu*r*uq),  Aq = wq*diag(gamma) (q pre-scaled
    by 1/sqrt(c)), uq = wq@gamma, cq = bq + wq@beta; same for k. The V-path
    affine is folded all the way into the P-projection eviction:
        out = x + r*(Wp @ attn @ (Av x)^T) + (Wp @ dv + bp), dv = cv - mu*r*uv
    (softmax rows sum to 1, so the V bias passes through attention as a
    per-channel constant). mu, r=rstd computed on device (AllReduce of
    per-batch sum/sumsq across the 4 cores of each batch).
  - Host re-lays the shard w-major: [8 h-rows][512 c][32 w * 64 t], so one
    attention group (8 w-locations) is a contiguous 512-column slice.
  - Locations are processed in PAIRS sharing the 128-wide stationary
    operand: VT for 2 locations in one matmul (x-pair stationary, Wv
    moving), scores S^T = K^T Q per pair as one [128,128] matmul (cross
    terms masked off with the causal mask), and AV as a full-K [128,128]
    matmul per pair (masked zeros in attn kill the cross contributions).
  - No max-subtraction in softmax (scores are O(1)); causal+pair mask is a
    0/1 multiply after exp; normalization by 1/rowsum via ones-matmul
    reductions/broadcasts on the PE.
"""

import numpy as np
import ml_dtypes

import concourse.bass as bass
import concourse.tile as tile
from concourse import bacc, mybir
from concourse.bass_utils import run_bass_kernel_spmd

P = 128
B, C, T, H, W = 2, 512, 64, 32, 32
NCORES = 8
HSH = H // 4          # 8 h-rows per core
CCH = C // P          # 4 c chunks
GRP = 8               # locations per attention group
NGRP = W // GRP       # 4 groups per h-row block
NPR = GRP // 2        # 4 location-pairs per group
WT = W * T            # 2048 free columns per (h-row, c) plane
EPS = 1e-6

f32 = mybir.dt.float32
bf16 = mybir.dt.bfloat16
AX = mybir.AxisListType.X
ALU = mybir.AluOpType
AF = mybir.ActivationFunctionType
BF = ml_dtypes.bfloat16


def build_nc(num_cores=NCORES, nblk=HSH, norm_n=None, replica_groups=None,
             use_collective=True):
    if norm_n is None:
        norm_n = C * T * H * W
    if replica_groups is None:
        replica_groups = [[0, 1, 2, 3], [4, 5, 6, 7]]
    nc = bacc.Bacc("TRN2", target_bir_lowering=False, debug=False,
                   num_devices=num_cores)

    xs = nc.declare_dram_parameter("xs", [nblk, C, WT], bf16, isOutput=False)
    wts = {}
    for nm in ("q", "k", "v", "p"):
        wts[nm] = nc.declare_dram_parameter(f"w{nm}t", [C, C], bf16,
                                            isOutput=False)
    ucol = nc.declare_dram_parameter("ucol", [P, 3 * CCH], f32, isOutput=False)
    ccol = nc.declare_dram_parameter("ccol", [P, 3 * CCH], f32, isOutput=False)
    bpcol = nc.declare_dram_parameter("bpcol", [P, CCH], f32, isOutput=False)
    maskp = nc.declare_dram_parameter("maskt", [P, NPR * P], bf16,
                                      isOutput=False)
    ones_row_f = nc.declare_dram_parameter("ones_row_f", [1, P], f32,
                                           isOutput=False)
    ones_col_b = nc.declare_dram_parameter("ones_col_b", [P, 1], bf16,
                                           isOutput=False)
    ones_mat_b = nc.declare_dram_parameter("ones_mat_b", [P, P], bf16,
                                           isOutput=False)
    outp = nc.declare_dram_parameter("out", [nblk, C, WT], f32, isOutput=True)
    cc_in = nc.dram_tensor("cc_in", [1, 2], f32)
    cc_out = nc.dram_tensor("cc_out", [1, 2], f32)

    with tile.TileContext(nc) as tc:
        with (
            tc.tile_pool(name="const", bufs=1) as const,
            tc.tile_pool(name="scal", bufs=1) as sc,
            tc.tile_pool(name="statp", bufs=4) as statp,
            tc.tile_pool(name="sqp", bufs=2) as sqp,
            tc.tile_pool(name="xpool", bufs=2) as xpool,
            tc.tile_pool(name="gpool", bufs=8) as gpool,
            tc.tile_pool(name="spool", bufs=2) as spool,
            tc.tile_pool(name="opool", bufs=4) as opool,
            tc.tile_pool(name="pp", bufs=3, space="PSUM") as pp,
            tc.tile_pool(name="pss", bufs=2, space="PSUM") as pss,
            tc.tile_pool(name="scp", bufs=2, space="PSUM") as scp,
            tc.tile_pool(name="psm", bufs=1, space="PSUM") as psm,
        ):
            # ---------- constants ----------
            w_sb = {}
            for nm in ("q", "k", "v", "p"):
                for ci in range(CCH):
                    t = const.tile([P, C], bf16, tag=f"w{nm}{ci}")
                    nc.sync.dma_start(t[:], wts[nm][ci * P:(ci + 1) * P, :])
                    w_sb[nm, ci] = t
            ucol_sb = const.tile([P, 3 * CCH], f32, tag="ucol")
            nc.sync.dma_start(ucol_sb[:], ucol[:])
            ccol_sb = const.tile([P, 3 * CCH], f32, tag="ccol")
            nc.sync.dma_start(ccol_sb[:], ccol[:])
            bpcol_sb = const.tile([P, CCH], f32, tag="bpcol")
            nc.sync.dma_start(bpcol_sb[:], bpcol[:])
            mask_sb = const.tile([P, NPR * P], bf16, tag="maskt")
            nc.sync.dma_start(mask_sb[:], maskp[:])
            ocb_sb = const.tile([P, 1], bf16, tag="ocb")
            nc.sync.dma_start(ocb_sb[:], ones_col_b[:])
            orf_sb = const.tile([1, P], f32, tag="orf")
            nc.sync.dma_start(orf_sb[:], ones_row_f[:])
            omb_sb = const.tile([P, P], bf16, tag="omb")
            nc.sync.dma_start(omb_sb[:], ones_mat_b[:])

            # ---------- stats (sum / sumsq over the whole shard) ----------
            # x-sum via ones-matmuls accumulating in one PSUM bank (PE is
            # idle during the stats phase); sumsq fused into the Square
            # activation's accum_out — DVE does almost no stats work
            nst = nblk * CCH // 2          # 1 MiB stats tiles (2 c-chunks)
            ssq = sc.tile([P, nst], f32, tag="ssq")
            ps_sum1 = psm.tile([1, 512], f32, tag="psm")
            for blk in range(nblk):
                for c2 in range(CCH // 2):
                    xt = statp.tile([P, 2 * WT], bf16, tag="xstat")
                    src = xs[blk, c2 * 2 * P:(c2 + 1) * 2 * P, :].rearrange(
                        "(a p) w -> p a w", p=P)
                    nc.sync.dma_start(xt[:].rearrange("p (a w) -> p a w", a=2),
                                      src)
                    i = blk * (CCH // 2) + c2
                    for j in range(2 * WT // 512):
                        nc.tensor.matmul(
                            ps_sum1[:], ocb_sb[:],
                            xt[:, j * 512:(j + 1) * 512],
                            start=(i == 0 and j == 0),
                            stop=(i == nst - 1 and j == 2 * WT // 512 - 1),
                            skip_group_check=True)
                    sq = sqp.tile([P, 2 * WT], bf16, tag="sq")
                    if i % 3 == 2:
                        # spread the square+reduce work across DVE too so the
                        # stats phase isn't paced by ScalarE alone
                        nc.vector.tensor_mul(sq[:], xt[:], xt[:])
                        nc.vector.reduce_sum(out=ssq[:, i:i + 1], in_=sq[:],
                                             axis=AX)
                    else:
                        nc.scalar.activation(sq[:], xt[:], AF.Square,
                                             accum_out=ssq[:, i:i + 1])
            st_sb = sc.tile([1, 2], f32, tag="st_sb")
            nc.vector.reduce_sum(out=st_sb[0:1, 0:1], in_=ps_sum1[:], axis=AX)
            nc.gpsimd.reduce_sum(out=st_sb[0:1, 1:2], in_=ssq[:],
                                 axis=mybir.AxisListType.XYZWC)
            nc.gpsimd.dma_start(cc_in[:], st_sb[:])
            if use_collective:
                nc.gpsimd.collective_compute(
                    "AllReduce", ALU.add, replica_groups=replica_groups,
                    ins=[cc_in[:]], outs=[cc_out[:]])
            else:
                nc.gpsimd.dma_start(cc_out[:], cc_in[:])
            stg = sc.tile([1, 2], f32, tag="stg")
            nc.gpsimd.dma_start(stg[:], cc_out[:])

            mean = sc.tile([1, 1], f32, tag="mean")
            nc.scalar.activation(mean[:], stg[:, 0:1], AF.Copy,
                                 bias=0.0, scale=1.0 / norm_n)
            ex2 = sc.tile([1, 1], f32, tag="ex2")
            nc.scalar.activation(ex2[:], stg[:, 1:2], AF.Copy,
                                 bias=0.0, scale=1.0 / norm_n)
            msq = sc.tile([1, 1], f32, tag="msq")
            nc.scalar.activation(msq[:], mean[:], AF.Square)
            varp = sc.tile([1, 1], f32, tag="varp")
            nc.vector.tensor_scalar(varp[:], ex2[:], msq[:], EPS,
                                    ALU.subtract, ALU.add)
            sqv = sc.tile([1, 1], f32, tag="sqv")      # = 1/rstd
            nc.scalar.activation(sqv[:], varp[:], AF.Sqrt)
            rst = sc.tile([1, 1], f32, tag="rst")      # = rstd
            nc.vector.reciprocal(rst[:], sqv[:])
            rmu = sc.tile([1, 1], f32, tag="rmu")      # = rstd*mean
            nc.vector.tensor_scalar(rmu[:], mean[:], rst[:], None, ALU.mult)
            vals = sc.tile([1, 3], f32, tag="vals")
            nc.vector.tensor_copy(vals[:, 0:1], rst[:])
            nc.vector.tensor_copy(vals[:, 1:2], rmu[:])
            nc.vector.tensor_copy(vals[:, 2:3], sqv[:])
            # broadcast (rstd, rstd*mean, 1/rstd) across partitions (K=1 mm)
            ps_b = psm.tile([P, 512], f32, tag="psm")
            nc.tensor.matmul(ps_b[:, 0:3], orf_sb[:], vals[:],
                             start=True, stop=True)
            rb = sc.tile([P, 3], f32, tag="rb")
            nc.vector.tensor_copy(rb[:], ps_b[:, 0:3])
            # all-(1/r) stationary for the softmax denominator matmul: the
            # rowsum matmul then directly yields Z/r, whose reciprocal is the
            # r/Z factor applied at the AV eviction
            oiv = sc.tile([P, P], bf16, tag="oiv")
            nc.vector.tensor_scalar(oiv[:], omb_sb[:], rb[:, 2:3], None,
                                    ALU.mult)
            # per-(proj,chunk) eviction biases for q,k,v: D = ccol - rmu*ucol
            dcol = sc.tile([P, 3 * CCH], f32, tag="dcol")
            nc.vector.tensor_scalar(dcol[:], ucol_sb[:], rb[:, 1:2], None,
                                    ALU.mult)
            nc.vector.tensor_sub(dcol[:], ccol_sb[:], dcol[:])
            # dp = Wp @ dv + bp  (per-channel constant added at P-eviction)
            dvb = sc.tile([P, CCH], bf16, tag="dvb")
            nc.vector.tensor_copy(dvb[:], dcol[:, 2 * CCH:3 * CCH])
            ps_dp = psm.tile([P, 512], f32, tag="psm")
            for co in range(CCH):
                for ci in range(CCH):
                    nc.tensor.matmul(
                        ps_dp[:, co:co + 1],
                        w_sb["p", ci][:, co * P:(co + 1) * P],
                        dvb[:, ci:ci + 1], start=(co == 0 and ci == 0),
                        stop=(ci == CCH - 1), skip_group_check=True)
            dp = sc.tile([P, CCH], f32, tag="dp")
            nc.vector.tensor_add(dp[:], ps_dp[:, 0:CCH], bpcol_sb[:])

            # ---------- main blocks ----------
            # One-group software pipeline: stage 1 of group g (projections,
            # scores, exp+mask) is emitted before stage 2 of group g-1
            # (rowsum, AV, P, out), so the softmax latency of g hides under
            # the projection matmuls of g and AV/P of g-1 — the in-order PE
            # queue never waits on ScalarE/DVE.

            def stage1(xb, cs):
                # Q, K projections: psum[co, (w,t)] over ci
                qk = {}
                for pi, nm in enumerate(("q", "k")):
                    for co in range(CCH):
                        ps = pp.tile([P, 512], f32, tag="pp")
                        for ci in range(CCH):
                            nc.tensor.matmul(
                                ps[:], w_sb[nm, ci][:, co * P:(co + 1) * P],
                                xb[ci][:, cs:cs + 512], start=(ci == 0),
                                stop=(ci == CCH - 1))
                        t = gpool.tile([P, 512], bf16, tag=f"{nm}g")
                        d = pi * CCH + co
                        # affine eviction on ScalarE (closer to PSUM, keeps
                        # DVE free): t = ps*r + dcol
                        nc.scalar.activation(
                            t[:], ps[:], AF.Identity,
                            bias=dcol[:, d:d + 1], scale=rb[:, 0:1])
                        qk[nm, co] = t

                # VT (raw): per loc PAIR, [128 (2w,s), 512 co]
                vtp = []
                for p in range(NPR):
                    ps = pss.tile([P, 512], f32, tag="ppv")
                    for ci in range(CCH):
                        nc.tensor.matmul(
                            ps[:], xb[ci][:, cs + p * P:cs + (p + 1) * P],
                            w_sb["v", ci][:], start=(ci == 0),
                            stop=(ci == CCH - 1))
                    t = gpool.tile([P, 512], bf16, tag="vtg")
                    nc.scalar.copy(t[:], ps[:])
                    vtp.append(t)

                # scores S^T[(2w,s), (2w,t)] per pair; 4 pair-chains share
                # one PSUM bank: the very first matmul start=True zeroes the
                # bank, later chains' first matmuls overwrite (has_written
                # cleared) and accumulate over ci.
                ps_s = scp.tile([P, 512], f32, tag="pss")
                for p in range(NPR):
                    for ci in range(CCH):
                        nc.tensor.matmul(
                            ps_s[:, p * P:(p + 1) * P],
                            qk["k", ci][:, p * P:(p + 1) * P],
                            qk["q", ci][:, p * P:(p + 1) * P],
                            start=(p == 0 and ci == 0),
                            stop=(ci == CCH - 1), skip_group_check=True)
                # unnormalized masked softmax numerator (normalization is
                # folded into the AV eviction as r/Z)
                pexp = spool.tile([P, 512], bf16, tag="pexp")
                nc.scalar.activation(pexp[:], ps_s[:], AF.Exp)
                pm = spool.tile([P, 512], bf16, tag="pmask")
                nc.vector.tensor_mul(pm[:], pexp[:], mask_sb[:])
                return vtp, pm

            def stage2(st):
                xb, cs, blk, vtp, pm = st
                # rowsum matmul with all-(1/r) stationary => Z/r, broadcast
                # across partitions; fast-approx reciprocal gives r/Z with
                # ~18 good bits, plenty for the bf16 og tiles
                ps_sum = psm.tile([P, 512], f32, tag="psm")
                nc.tensor.matmul(ps_sum[:], oiv[:], pm[:],
                                 start=True, stop=True)
                rz = spool.tile([P, 512], f32, tag="rz")
                nc.vector.reciprocal_approx_fast(out=rz[:], in_=ps_sum[:])

                # AV: O[c,(2w,t)] per pair, full-K (mask zeros kill the
                # cross-location contributions); eviction applies r/Z
                og = []
                for ch in range(CCH):
                    ps_o = pp.tile([P, 512], f32, tag="pp")
                    for p in range(NPR):
                        nc.tensor.matmul(
                            ps_o[:, p * P:(p + 1) * P],
                            vtp[p][:, ch * P:(ch + 1) * P],
                            pm[:, p * P:(p + 1) * P],
                            start=(p == 0), stop=True,
                            skip_group_check=True)
                    t = gpool.tile([P, 512], bf16, tag="og")
                    nc.vector.tensor_mul(t[:], ps_o[:], rz[:])
                    og.append(t)

                # P-projection + bias + residual
                for co in range(CCH):
                    ps = pp.tile([P, 512], f32, tag="pp")
                    for ci in range(CCH):
                        nc.tensor.matmul(
                            ps[:], w_sb["p", ci][:, co * P:(co + 1) * P],
                            og[ci][:], start=(ci == 0),
                            stop=(ci == CCH - 1))
                    slab = opool.tile([P, 512], f32, tag="oslab")
                    nc.vector.tensor_scalar(
                        slab[:], ps[:], dp[:, co:co + 1], None, ALU.add)
                    nc.vector.tensor_add(slab[:], slab[:],
                                         xb[co][:, cs:cs + 512])
                    nc.sync.dma_start(
                        outp[blk, co * P:(co + 1) * P, cs:cs + 512],
                        slab[:])

            pending = None
            for blk in range(nblk):
                xb = []
                for ci in range(CCH):
                    t = xpool.tile([P, WT], bf16, tag=f"xb{ci}")
                    nc.sync.dma_start(t[:], xs[blk, ci * P:(ci + 1) * P, :])
                    xb.append(t)
                for g in range(NGRP):
                    cs = g * GRP * T          # 512-col slice of this group
                    vtp, pm = stage1(xb, cs)
                    if pending is not None:
                        stage2(pending)
                    pending = (xb, cs, blk, vtp, pm)
            stage2(pending)
    nc.compile()
    return nc


def host_prep(gamma, beta, wq, bq, wk, bk, wv, bv, wp, bp):
    """Fold gamma/beta into weights; build all constant tensors."""
    s = 1.0 / np.sqrt(np.float32(C))
    g = gamma.astype(np.float64)

    def fold(w, bias, scale):
        a = (w.astype(np.float64) * g[None, :]) * scale      # (co, ci)
        u = (w.astype(np.float64) @ g) * scale               # (co,)
        c0 = (bias.astype(np.float64) + w.astype(np.float64) @
              beta.astype(np.float64)) * scale
        return (np.ascontiguousarray(a.T).astype(BF),
                u.astype(np.float32), c0.astype(np.float32))

    aqt, uq, cq = fold(wq, bq, s)
    akt, uk, ck = fold(wk, bk, 1.0)
    avt, uv, cv = fold(wv, bv, 1.0)
    apt = np.ascontiguousarray(wp.T.astype(np.float32)).astype(BF)

    ucol = np.empty((P, 3 * CCH), np.float32)
    ccol = np.empty((P, 3 * CCH), np.float32)
    for pi, (u, c0) in enumerate(((uq, cq), (uk, ck), (uv, cv))):
        for ch in range(CCH):
            ucol[:, pi * CCH + ch] = u[ch * P:(ch + 1) * P]
            ccol[:, pi * CCH + ch] = c0[ch * P:(ch + 1) * P]
    bpcol = np.empty((P, CCH), np.float32)
    for ch in range(CCH):
        bpcol[:, ch] = bp[ch * P:(ch + 1) * P]

    # pair mask [128, 4*128]: diag 64x64 halves get causal triu (s<=t),
    # off-diag (cross-location) halves are zero; identical per pair.
    tri = np.triu(np.ones((T, T), np.float32))
    blkm = np.zeros((P, P), np.float32)
    blkm[0:T, 0:T] = tri
    blkm[T:2 * T, T:2 * T] = tri
    maskt = np.tile(blkm, (1, NPR))

    consts = {
        "wqt": aqt, "wkt": akt, "wvt": avt, "wpt": apt,
        "ucol": ucol, "ccol": ccol, "bpcol": bpcol,
        "maskt": maskt.astype(BF),
        "ones_row_f": np.ones((1, P), np.float32),
        "ones_col_b": np.ones((P, 1), BF),
        "ones_mat_b": np.ones((P, P), BF),
    }
    return consts


_NC_CACHE = {}


def kernel(x, gamma, beta, wq, bq, wk, bk, wv, bv, wp, bp):
    x = np.asarray(x, np.float32)
    args = [np.asarray(a, np.float32) for a in
            (gamma, beta, wq, bq, wk, bk, wv, bv, wp, bp)]
    consts = host_prep(*args)

    if "nc" not in _NC_CACHE:
        _NC_CACHE["nc"] = build_nc()
    nc = _NC_CACHE["nc"]

    in_maps = []
    for core in range(NCORES):
        b, hg = core // 4, core % 4
        shard = x[b, :, :, hg * HSH:(hg + 1) * HSH, :]        # (C,T,HSH,W)
        shard = np.ascontiguousarray(
            shard.transpose(2, 0, 3, 1)).reshape(HSH, C, WT)  # w-major
        in_maps.append({"xs": shard.astype(BF), **consts})

    global _last_in_maps
    _last_in_maps = in_maps
    res = run_bass_kernel_spmd(nc, in_maps, list(range(NCORES)))

    out = np.empty((B, C, T, H, W), np.float32)
    for core in range(NCORES):
        b, hg = core // 4, core % 4
        o = res.results[core]["out"].reshape(HSH, C, W, T)
        out[b, :, :, hg * HSH:(hg + 1) * HSH, :] = o.transpose(1, 3, 0, 2)
    return out


# revision 34
# speedup vs baseline: 1.1775x; 1.0787x over previous
"""CausalTemporalAttnBlock Trainium2 kernel.

Problem: out = x + Wp @ attn(norm(x)) + bp, where norm is GroupNorm(1 group)
over (c,t,h,w) per batch, attention is causal over t, independent per (b,h,w).
Shapes: x (2, 512, 64, 32, 32) fp32; four (512,512) weights + biases.

Strategy (8 NeuronCores, zero communication except a 8-byte AllReduce for
the GroupNorm stats):
  - core i handles batch i//4, h-rows [8*(i%4), 8*(i%4)+8), all w: 256 (h,w)
    locations per core.
  - Whole matmul datapath in bf16 (fp32 PSUM accumulation): full-rate PE
    streaming (fp32 is half rate) and fast weight load. Host quantizes x
    and the folded weights to bf16 (RNE); error stays ~1e-3 vs the 2e-2
    budget.
  - Host folds gamma/beta into the projection weights:
        q = r*(Aq @ x) + (cq - mu*r*uq),  Aq = wq*diag(gamma) (q pre-scaled
    by 1/sqrt(c)), uq = wq@gamma, cq = bq + wq@beta; same for k. The V-path
    affine is folded all the way into the P-projection eviction:
        out = x + r*(Wp @ attn @ (Av x)^T) + (Wp @ dv + bp), dv = cv - mu*r*uv
    (softmax rows sum to 1, so the V bias passes through attention as a
    per-channel constant). mu, r=rstd computed on device (AllReduce of
    per-batch sum/sumsq across the 4 cores of each batch).
  - Host re-lays the shard w-major: [8 h-rows][512 c][32 w * 64 t], so one
    attention group (8 w-locations) is a contiguous 512-column slice.
  - Locations are processed in PAIRS sharing the 128-wide stationary
    operand: VT for 2 locations in one matmul (x-pair stationary, Wv
    moving), scores S^T = K^T Q per pair as one [128,128] matmul (cross
    terms masked off with the causal mask), and AV as a full-K [128,128]
    matmul per pair (masked zeros in attn kill the cross contributions).
  - No max-subtraction in softmax (scores are O(1)); causal+pair mask is a
    0/1 multiply after exp; normalization by 1/rowsum via ones-matmul
    reductions/broadcasts on the PE.
"""

import numpy as np
import ml_dtypes

import concourse.bass as bass
import concourse.tile as tile
from concourse import bacc, mybir
from concourse.bass_utils import run_bass_kernel_spmd

P = 128
B, C, T, H, W = 2, 512, 64, 32, 32
NCORES = 8
HSH = H // 4          # 8 h-rows per core
CCH = C // P          # 4 c chunks
GRP = 8               # locations per attention group
NGRP = W // GRP       # 4 groups per h-row block
NPR = GRP // 2        # 4 location-pairs per group
WT = W * T            # 2048 free columns per (h-row, c) plane
EPS = 1e-6

f32 = mybir.dt.float32
bf16 = mybir.dt.bfloat16
AX = mybir.AxisListType.X
ALU = mybir.AluOpType
AF = mybir.ActivationFunctionType
BF = ml_dtypes.bfloat16


def build_nc(num_cores=NCORES, nblk=HSH, norm_n=None, replica_groups=None,
             use_collective=True):
    if norm_n is None:
        norm_n = C * T * H * W
    if replica_groups is None:
        replica_groups = [[0, 1, 2, 3], [4, 5, 6, 7]]
    nc = bacc.Bacc("TRN2", target_bir_lowering=False, debug=False,
                   num_devices=num_cores)

    xs = nc.declare_dram_parameter("xs", [nblk, C, WT], bf16, isOutput=False)
    wts = {}
    for nm in ("y", "v", "p"):
        wts[nm] = nc.declare_dram_parameter(f"w{nm}t", [C, C], bf16,
                                            isOutput=False)
    ucol = nc.declare_dram_parameter("ucol", [P, CCH], f32, isOutput=False)
    ccol = nc.declare_dram_parameter("ccol", [P, CCH], f32, isOutput=False)
    w1col = nc.declare_dram_parameter("w1col", [P, CCH], f32, isOutput=False)
    w2col = nc.declare_dram_parameter("w2col", [P, CCH], f32, isOutput=False)
    bpcol = nc.declare_dram_parameter("bpcol", [P, CCH], f32, isOutput=False)
    maskp = nc.declare_dram_parameter("maskt", [P, NPR * P], bf16,
                                      isOutput=False)
    ones_row_f = nc.declare_dram_parameter("ones_row_f", [1, P], f32,
                                           isOutput=False)
    ones_col_b = nc.declare_dram_parameter("ones_col_b", [P, 1], bf16,
                                           isOutput=False)
    ones_row_b = nc.declare_dram_parameter("ones_row_b", [1, P], bf16,
                                           isOutput=False)
    ones_mat_b = nc.declare_dram_parameter("ones_mat_b", [P, P], bf16,
                                           isOutput=False)
    outp = nc.declare_dram_parameter("out", [nblk, C, WT], f32, isOutput=True)
    cc_in = nc.dram_tensor("cc_in", [1, 2], f32)
    cc_out = nc.dram_tensor("cc_out", [1, 2], f32)

    with tile.TileContext(nc) as tc:
        with (
            tc.tile_pool(name="const", bufs=1) as const,
            tc.tile_pool(name="scal", bufs=1) as sc,
            tc.tile_pool(name="statp", bufs=4) as statp,
            tc.tile_pool(name="sqp", bufs=2) as sqp,
            tc.tile_pool(name="xpool", bufs=2) as xpool,
            tc.tile_pool(name="gpool", bufs=8) as gpool,
            tc.tile_pool(name="spool", bufs=2) as spool,
            tc.tile_pool(name="opool", bufs=4) as opool,
            tc.tile_pool(name="pp", bufs=3, space="PSUM") as pp,
            tc.tile_pool(name="pss", bufs=2, space="PSUM") as pss,
            tc.tile_pool(name="scp", bufs=2, space="PSUM") as scp,
            tc.tile_pool(name="psm", bufs=1, space="PSUM") as psm,
        ):
            # ---------- constants ----------
            w_sb = {}
            for nm in ("y", "v", "p"):
                for ci in range(CCH):
                    t = const.tile([P, C], bf16, tag=f"w{nm}{ci}")
                    nc.sync.dma_start(t[:], wts[nm][ci * P:(ci + 1) * P, :])
                    w_sb[nm, ci] = t
            ucol_sb = const.tile([P, CCH], f32, tag="ucol")
            nc.sync.dma_start(ucol_sb[:], ucol[:])
            ccol_sb = const.tile([P, CCH], f32, tag="ccol")
            nc.sync.dma_start(ccol_sb[:], ccol[:])
            w1_sb = const.tile([P, CCH], f32, tag="w1col")
            nc.sync.dma_start(w1_sb[:], w1col[:])
            w2_sb = const.tile([P, CCH], f32, tag="w2col")
            nc.sync.dma_start(w2_sb[:], w2col[:])
            bpcol_sb = const.tile([P, CCH], f32, tag="bpcol")
            nc.sync.dma_start(bpcol_sb[:], bpcol[:])
            mask_sb = const.tile([P, NPR * P], bf16, tag="maskt")
            nc.sync.dma_start(mask_sb[:], maskp[:])
            ocb_sb = const.tile([P, 1], bf16, tag="ocb")
            nc.sync.dma_start(ocb_sb[:], ones_col_b[:])
            orb_sb = const.tile([1, P], bf16, tag="orb")
            nc.sync.dma_start(orb_sb[:], ones_row_b[:])
            orf_sb = const.tile([1, P], f32, tag="orf")
            nc.sync.dma_start(orf_sb[:], ones_row_f[:])
            omb_sb = const.tile([P, P], bf16, tag="omb")
            nc.sync.dma_start(omb_sb[:], ones_mat_b[:])

            # ---------- stats (sum / sumsq over the whole shard) ----------
            # x-sum via ones-matmuls accumulating in one PSUM bank (PE is
            # idle during the stats phase); sumsq fused into the Square
            # activation's accum_out — DVE does almost no stats work
            nst = nblk * CCH // 2          # 1 MiB stats tiles (2 c-chunks)
            ssq = sc.tile([P, nst], f32, tag="ssq")
            ps_sum1 = psm.tile([1, 512], f32, tag="psm")
            for blk in range(nblk):
                for c2 in range(CCH // 2):
                    xt = statp.tile([P, 2 * WT], bf16, tag="xstat")
                    src = xs[blk, c2 * 2 * P:(c2 + 1) * 2 * P, :].rearrange(
                        "(a p) w -> p a w", p=P)
                    nc.sync.dma_start(xt[:].rearrange("p (a w) -> p a w", a=2),
                                      src)
                    i = blk * (CCH // 2) + c2
                    for j in range(2 * WT // 512):
                        nc.tensor.matmul(
                            ps_sum1[:], ocb_sb[:],
                            xt[:, j * 512:(j + 1) * 512],
                            start=(i == 0 and j == 0),
                            stop=(i == nst - 1 and j == 2 * WT // 512 - 1),
                            skip_group_check=True)
                    sq = sqp.tile([P, 2 * WT], bf16, tag="sq")
                    if i % 3 == 2:
                        # spread the square+reduce work across DVE too so the
                        # stats phase isn't paced by ScalarE alone
                        nc.vector.tensor_mul(sq[:], xt[:], xt[:])
                        nc.vector.reduce_sum(out=ssq[:, i:i + 1], in_=sq[:],
                                             axis=AX)
                    else:
                        nc.scalar.activation(sq[:], xt[:], AF.Square,
                                             accum_out=ssq[:, i:i + 1])
            st_sb = sc.tile([1, 2], f32, tag="st_sb")
            nc.vector.reduce_sum(out=st_sb[0:1, 0:1], in_=ps_sum1[:], axis=AX)
            nc.gpsimd.reduce_sum(out=st_sb[0:1, 1:2], in_=ssq[:],
                                 axis=mybir.AxisListType.XYZWC)
            nc.gpsimd.dma_start(cc_in[:], st_sb[:])
            if use_collective:
                nc.gpsimd.collective_compute(
                    "AllReduce", ALU.add, replica_groups=replica_groups,
                    ins=[cc_in[:]], outs=[cc_out[:]])
            else:
                nc.gpsimd.dma_start(cc_out[:], cc_in[:])
            stg = sc.tile([1, 2], f32, tag="stg")
            nc.gpsimd.dma_start(stg[:], cc_out[:])

            mean = sc.tile([1, 1], f32, tag="mean")
            nc.scalar.activation(mean[:], stg[:, 0:1], AF.Copy,
                                 bias=0.0, scale=1.0 / norm_n)
            ex2 = sc.tile([1, 1], f32, tag="ex2")
            nc.scalar.activation(ex2[:], stg[:, 1:2], AF.Copy,
                                 bias=0.0, scale=1.0 / norm_n)
            msq = sc.tile([1, 1], f32, tag="msq")
            nc.scalar.activation(msq[:], mean[:], AF.Square)
            varp = sc.tile([1, 1], f32, tag="varp")
            nc.vector.tensor_scalar(varp[:], ex2[:], msq[:], EPS,
                                    ALU.subtract, ALU.add)
            sqv = sc.tile([1, 1], f32, tag="sqv")      # = 1/rstd
            nc.scalar.activation(sqv[:], varp[:], AF.Sqrt)
            rst = sc.tile([1, 1], f32, tag="rst")      # = rstd
            nc.vector.reciprocal(rst[:], sqv[:])
            rmu = sc.tile([1, 1], f32, tag="rmu")      # = rstd*mean
            nc.vector.tensor_scalar(rmu[:], mean[:], rst[:], None, ALU.mult)
            rsq = sc.tile([1, 1], f32, tag="rsq")  # = rstd^2
            nc.vector.tensor_scalar(rsq[:], rst[:], rst[:], None, ALU.mult)
            vals = sc.tile([1, 4], f32, tag="vals")
            nc.vector.tensor_copy(vals[:, 0:1], rst[:])
            nc.vector.tensor_copy(vals[:, 1:2], rmu[:])
            nc.vector.tensor_copy(vals[:, 2:3], sqv[:])
            nc.vector.tensor_copy(vals[:, 3:4], rsq[:])
            # broadcast (rstd, rstd*mean, 1/rstd, rstd^2) across partitions
            ps_b = psm.tile([P, 512], f32, tag="psm")
            nc.tensor.matmul(ps_b[:, 0:4], orf_sb[:], vals[:],
                             start=True, stop=True)
            rb = sc.tile([P, 4], f32, tag="rb")
            nc.vector.tensor_copy(rb[:], ps_b[:, 0:4])
            # all-(1/r) stationary for the softmax denominator matmul: the
            # rowsum matmul then directly yields Z/r, whose reciprocal is the
            # r/Z factor applied at the AV eviction
            oiv = sc.tile([P, P], bf16, tag="oiv")
            nc.vector.tensor_scalar(oiv[:], omb_sb[:], rb[:, 2:3], None,
                                    ALU.mult)
            # v-projection eviction bias: dv = ccol - rmu*ucol
            dcol = sc.tile([P, CCH], f32, tag="dcol")
            nc.vector.tensor_scalar(dcol[:], ucol_sb[:], rb[:, 1:2], None,
                                    ALU.mult)
            nc.vector.tensor_sub(dcol[:], ccol_sb[:], dcol[:])
            # score rank-1 vector: v0 = (w1 - rmu*w2)/rstd, so that after the
            # exp's r^2 scale the surviving affine term is r*(Kr^T dq)[s]
            v0c = sc.tile([P, CCH], f32, tag="v0c")
            nc.vector.tensor_scalar(v0c[:], w2_sb[:], rb[:, 1:2], None,
                                    ALU.mult)
            nc.vector.tensor_sub(v0c[:], w1_sb[:], v0c[:])
            v0b = sc.tile([P, CCH], bf16, tag="v0b")
            nc.vector.tensor_scalar(v0b[:], v0c[:], rb[:, 2:3], None,
                                    ALU.mult)
            # dp = Wp @ dv + bp  (per-channel constant added at P-eviction)
            dvb = sc.tile([P, CCH], bf16, tag="dvb")
            nc.vector.tensor_copy(dvb[:], dcol[:])
            ps_dp = psm.tile([P, 512], f32, tag="psm")
            for co in range(CCH):
                for ci in range(CCH):
                    nc.tensor.matmul(
                        ps_dp[:, co:co + 1],
                        w_sb["p", ci][:, co * P:(co + 1) * P],
                        dvb[:, ci:ci + 1], start=(co == 0 and ci == 0),
                        stop=(ci == CCH - 1), skip_group_check=True)
            dp = sc.tile([P, CCH], f32, tag="dp")
            nc.vector.tensor_add(dp[:], ps_dp[:, 0:CCH], bpcol_sb[:])

            # ---------- main blocks ----------
            # One-group software pipeline: stage 1 of group g (projections,
            # scores, exp+mask) is emitted before stage 2 of group g-1
            # (rowsum, AV, P, out), so the softmax latency of g hides under
            # the projection matmuls of g and AV/P of g-1 — the in-order PE
            # queue never waits on ScalarE/DVE.

            def stage1(xb, cs):
                # score rank-1 bias row h = x^T v0 for this group's columns
                # (cheap: 1-col stationary, N=512 streams)
                ps_h = psm.tile([1, 512], f32, tag="psm")
                for ci in range(CCH):
                    nc.tensor.matmul(ps_h[:], v0b[:, ci:ci + 1],
                                     xb[ci][:, cs:cs + 512],
                                     start=(ci == 0), stop=(ci == CCH - 1))
                hrow = spool.tile([1, 512], bf16, tag="hrow")
                with nc.allow_low_precision(
                        reason="bf16 score bias fine at 2e-2 target"):
                    nc.vector.tensor_copy(hrow[:], ps_h[:])

                # Y = (Ak^T Aq) x — the single projection that replaces both
                # Q and K: scores are the bilinear form x^T Y
                yg = []
                for co in range(CCH):
                    ps = pp.tile([P, 512], f32, tag="pp")
                    for ci in range(CCH):
                        nc.tensor.matmul(
                            ps[:], w_sb["y", ci][:, co * P:(co + 1) * P],
                            xb[ci][:, cs:cs + 512], start=(ci == 0),
                            stop=(ci == CCH - 1))
                    t = gpool.tile([P, 512], bf16, tag="yg")
                    nc.scalar.copy(t[:], ps[:])
                    yg.append(t)

                # VT (raw): per loc PAIR, [128 (2w,s), 512 co]
                vtp = []
                for p in range(NPR):
                    ps = pss.tile([P, 512], f32, tag="ppv")
                    for ci in range(CCH):
                        nc.tensor.matmul(
                            ps[:], xb[ci][:, cs + p * P:cs + (p + 1) * P],
                            w_sb["v", ci][:], start=(ci == 0),
                            stop=(ci == CCH - 1))
                    t = gpool.tile([P, 512], bf16, tag="vtg")
                    nc.scalar.copy(t[:], ps[:])
                    vtp.append(t)

                # scores S^T[(2w,s), (2w,t)] per pair = x_pair^T Y_pair,
                # plus the rank-1 h[s] x ones_t accumulated on top; 4
                # pair-chains share one PSUM bank: the very first matmul
                # start=True zeroes the bank, later chains' first matmuls
                # overwrite (has_written cleared) and accumulate.
                ps_s = scp.tile([P, 512], f32, tag="pss")
                for p in range(NPR):
                    for ci in range(CCH):
                        nc.tensor.matmul(
                            ps_s[:, p * P:(p + 1) * P],
                            xb[ci][:, cs + p * P:cs + (p + 1) * P],
                            yg[ci][:, p * P:(p + 1) * P],
                            start=(p == 0 and ci == 0),
                            stop=False, skip_group_check=True)
                    nc.tensor.matmul(
                        ps_s[:, p * P:(p + 1) * P],
                        hrow[:, p * P:(p + 1) * P], orb_sb[:],
                        start=False, stop=True, skip_group_check=True)
                # unnormalized masked softmax numerator; the exp's scale
                # applies the r^2 the bilinear form is missing
                # (normalization is folded into the AV eviction as r/Z)
                pexp = spool.tile([P, 512], bf16, tag="pexp")
                nc.scalar.activation(pexp[:], ps_s[:], AF.Exp,
                                     scale=rb[:, 3:4])
                pm = spool.tile([P, 512], bf16, tag="pmask")
                nc.vector.tensor_mul(pm[:], pexp[:], mask_sb[:])
                return vtp, pm

            def stage2(st):
                xb, cs, blk, vtp, pm = st
                # rowsum matmul with all-(1/r) stationary => Z/r, broadcast
                # across partitions; fast-approx reciprocal gives r/Z with
                # ~18 good bits, plenty for the bf16 og tiles
                ps_sum = psm.tile([P, 512], f32, tag="psm")
                nc.tensor.matmul(ps_sum[:], oiv[:], pm[:],
                                 start=True, stop=True)
                rz = spool.tile([P, 512], f32, tag="rz")
                nc.vector.reciprocal_approx_fast(out=rz[:], in_=ps_sum[:])

                # AV: O[c,(2w,t)] per pair, full-K (mask zeros kill the
                # cross-location contributions); eviction applies r/Z
                og = []
                for ch in range(CCH):
                    ps_o = pp.tile([P, 512], f32, tag="pp")
                    for p in range(NPR):
                        nc.tensor.matmul(
                            ps_o[:, p * P:(p + 1) * P],
                            vtp[p][:, ch * P:(ch + 1) * P],
                            pm[:, p * P:(p + 1) * P],
                            start=(p == 0), stop=True,
                            skip_group_check=True)
                    t = gpool.tile([P, 512], bf16, tag="og")
                    nc.vector.tensor_mul(t[:], ps_o[:], rz[:])
                    og.append(t)

                # P-projection + bias + residual
                for co in range(CCH):
                    ps = pp.tile([P, 512], f32, tag="pp")
                    for ci in range(CCH):
                        nc.tensor.matmul(
                            ps[:], w_sb["p", ci][:, co * P:(co + 1) * P],
                            og[ci][:], start=(ci == 0),
                            stop=(ci == CCH - 1))
                    slab = opool.tile([P, 512], f32, tag="oslab")
                    nc.vector.tensor_scalar(
                        slab[:], ps[:], dp[:, co:co + 1], None, ALU.add)
                    nc.vector.tensor_add(slab[:], slab[:],
                                         xb[co][:, cs:cs + 512])
                    nc.sync.dma_start(
                        outp[blk, co * P:(co + 1) * P, cs:cs + 512],
                        slab[:])

            pending = None
            for blk in range(nblk):
                xb = []
                for ci in range(CCH):
                    t = xpool.tile([P, WT], bf16, tag=f"xb{ci}")
                    nc.sync.dma_start(t[:], xs[blk, ci * P:(ci + 1) * P, :])
                    xb.append(t)
                for g in range(NGRP):
                    cs = g * GRP * T          # 512-col slice of this group
                    vtp, pm = stage1(xb, cs)
                    if pending is not None:
                        stage2(pending)
                    pending = (xb, cs, blk, vtp, pm)
            stage2(pending)
    nc.compile()
    return nc


def host_prep(gamma, beta, wq, bq, wk, bk, wv, bv, wp, bp):
    """Fold gamma/beta into weights; build all constant tensors."""
    s = 1.0 / np.sqrt(np.float64(C))
    g = gamma.astype(np.float64)

    def fold(w, bias, scale):
        a = (w.astype(np.float64) * g[None, :]) * scale      # (co, ci)
        u = (w.astype(np.float64) @ g) * scale               # (co,)
        c0 = (bias.astype(np.float64) + w.astype(np.float64) @
              beta.astype(np.float64)) * scale
        return a, u, c0

    aq, uq, cq = fold(wq, bq, s)
    ak, uk, ck = fold(wk, bk, 1.0)
    av, uv, cv = fold(wv, bv, 1.0)
    # scores are bilinear: S = (Ak x)^T (Aq x) = x^T G x with G = Ak^T Aq;
    # the surviving affine term (s-dependent only — t-terms cancel in
    # softmax) uses w1/w2: h = x^T Ak^T (cq - mu*r*uq)
    G = ak.T @ aq
    w1 = ak.T @ cq
    w2 = ak.T @ uq
    gyt = np.ascontiguousarray(G.T).astype(BF)
    avt = np.ascontiguousarray(av.T).astype(BF)
    apt = np.ascontiguousarray(wp.T.astype(np.float64)).astype(BF)

    def colize(v):
        out = np.empty((P, CCH), np.float32)
        for ch in range(CCH):
            out[:, ch] = v[ch * P:(ch + 1) * P]
        return out

    ucol = colize(uv)
    ccol = colize(cv)
    w1c = colize(w1)
    w2c = colize(w2)
    bpcol = colize(bp.astype(np.float64))

    # pair mask [128, 4*128]: diag 64x64 halves get causal triu (s<=t),
    # off-diag (cross-location) halves are zero; identical per pair.
    tri = np.triu(np.ones((T, T), np.float32))
    blkm = np.zeros((P, P), np.float32)
    blkm[0:T, 0:T] = tri
    blkm[T:2 * T, T:2 * T] = tri
    maskt = np.tile(blkm, (1, NPR))

    consts = {
        "wyt": gyt, "wvt": avt, "wpt": apt,
        "ucol": ucol, "ccol": ccol, "bpcol": bpcol,
        "w1col": w1c, "w2col": w2c,
        "maskt": maskt.astype(BF),
        "ones_row_f": np.ones((1, P), np.float32),
        "ones_col_b": np.ones((P, 1), BF),
        "ones_row_b": np.ones((1, P), BF),
        "ones_mat_b": np.ones((P, P), BF),
    }
    return consts


_NC_CACHE = {}


def kernel(x, gamma, beta, wq, bq, wk, bk, wv, bv, wp, bp):
    x = np.asarray(x, np.float32)
    args = [np.asarray(a, np.float32) for a in
            (gamma, beta, wq, bq, wk, bk, wv, bv, wp, bp)]
    consts = host_prep(*args)

    if "nc" not in _NC_CACHE:
        _NC_CACHE["nc"] = build_nc()
    nc = _NC_CACHE["nc"]

    in_maps = []
    for core in range(NCORES):
        b, hg = core // 4, core % 4
        shard = x[b, :, :, hg * HSH:(hg + 1) * HSH, :]        # (C,T,HSH,W)
        shard = np.ascontiguousarray(
            shard.transpose(2, 0, 3, 1)).reshape(HSH, C, WT)  # w-major
        in_maps.append({"xs": shard.astype(BF), **consts})

    global _last_in_maps
    _last_in_maps = in_maps
    res = run_bass_kernel_spmd(nc, in_maps, list(range(NCORES)))

    out = np.empty((B, C, T, H, W), np.float32)
    for core in range(NCORES):
        b, hg = core // 4, core % 4
        o = res.results[core]["out"].reshape(HSH, C, W, T)
        out[b, :, :, hg * HSH:(hg + 1) * HSH, :] = o.transpose(1, 3, 0, 2)
    return out


# revision 44
# speedup vs baseline: 1.1967x; 1.0163x over previous
"""CausalTemporalAttnBlock Trainium2 kernel.

Problem: out = x + Wp @ attn(norm(x)) + bp, where norm is GroupNorm(1 group)
over (c,t,h,w) per batch, attention is causal over t, independent per (b,h,w).
Shapes: x (2, 512, 64, 32, 32) fp32; four (512,512) weights + biases.

Strategy (8 NeuronCores, zero communication except a 8-byte AllReduce for
the GroupNorm stats):
  - core i handles batch i//4, h-rows [8*(i%4), 8*(i%4)+8), all w: 256 (h,w)
    locations per core.
  - Whole matmul datapath in bf16 (fp32 PSUM accumulation): full-rate PE
    streaming (fp32 is half rate) and fast weight load. Host quantizes x
    and the folded weights to bf16 (RNE); error stays ~1e-3 vs the 2e-2
    budget.
  - Host folds gamma/beta into the projection weights:
        q = r*(Aq @ x) + (cq - mu*r*uq),  Aq = wq*diag(gamma) (q pre-scaled
    by 1/sqrt(c)), uq = wq@gamma, cq = bq + wq@beta; same for k. The V-path
    affine is folded all the way into the P-projection eviction:
        out = x + r*(Wp @ attn @ (Av x)^T) + (Wp @ dv + bp), dv = cv - mu*r*uv
    (softmax rows sum to 1, so the V bias passes through attention as a
    per-channel constant). mu, r=rstd computed on device (AllReduce of
    per-batch sum/sumsq across the 4 cores of each batch).
  - Host re-lays the shard w-major: [8 h-rows][512 c][32 w * 64 t], so one
    attention group (8 w-locations) is a contiguous 512-column slice.
  - Locations are processed in PAIRS sharing the 128-wide stationary
    operand: VT for 2 locations in one matmul (x-pair stationary, Wv
    moving), scores S^T = K^T Q per pair as one [128,128] matmul (cross
    terms masked off with the causal mask), and AV as a full-K [128,128]
    matmul per pair (masked zeros in attn kill the cross contributions).
  - No max-subtraction in softmax (scores are O(1)); causal+pair mask is a
    0/1 multiply after exp; normalization by 1/rowsum via ones-matmul
    reductions/broadcasts on the PE.
"""

import numpy as np
import ml_dtypes

import concourse.bass as bass
import concourse.tile as tile
from concourse import bacc, mybir
from concourse.bass_utils import run_bass_kernel_spmd

P = 128
B, C, T, H, W = 2, 512, 64, 32, 32
NCORES = 8
HSH = H // 4          # 8 h-rows per core
CCH = C // P          # 4 c chunks
GRP = 8               # locations per attention group
NGRP = W // GRP       # 4 groups per h-row block
NPR = GRP // 2        # 4 location-pairs per group
WT = W * T            # 2048 free columns per (h-row, c) plane
EPS = 1e-6

f32 = mybir.dt.float32
bf16 = mybir.dt.bfloat16
AX = mybir.AxisListType.X
ALU = mybir.AluOpType
AF = mybir.ActivationFunctionType
BF = ml_dtypes.bfloat16


def build_nc(num_cores=NCORES, nblk=HSH, norm_n=None, replica_groups=None,
             use_collective=True):
    if norm_n is None:
        norm_n = C * T * H * W
    if replica_groups is None:
        replica_groups = [[0, 1, 2, 3], [4, 5, 6, 7]]
    nc = bacc.Bacc("TRN2", target_bir_lowering=False, debug=False,
                   num_devices=num_cores)

    xs = nc.declare_dram_parameter("xs", [nblk, C, WT], bf16, isOutput=False)
    wts = {}
    for nm in ("y", "v", "p"):
        wts[nm] = nc.declare_dram_parameter(f"w{nm}t", [C, C], bf16,
                                            isOutput=False)
    w1col = nc.declare_dram_parameter("w1col", [P, CCH], f32, isOutput=False)
    w2col = nc.declare_dram_parameter("w2col", [P, CCH], f32, isOutput=False)
    pv1col = nc.declare_dram_parameter("pv1col", [P, CCH], f32, isOutput=False)
    pv2col = nc.declare_dram_parameter("pv2col", [P, CCH], f32, isOutput=False)
    maskp = nc.declare_dram_parameter("maskt", [P, NPR * P], bf16,
                                      isOutput=False)
    ones_col_b = nc.declare_dram_parameter("ones_col_b", [P, 1], bf16,
                                           isOutput=False)
    ones_row_b = nc.declare_dram_parameter("ones_row_b", [1, P], bf16,
                                           isOutput=False)
    ones_mat_b = nc.declare_dram_parameter("ones_mat_b", [P, P], bf16,
                                           isOutput=False)
    outp = nc.declare_dram_parameter("out", [nblk, C, WT], f32, isOutput=True)
    cc_in = nc.dram_tensor("cc_in", [1, 2], f32)
    cc_out = nc.dram_tensor("cc_out", [1, 2], f32)

    with tile.TileContext(nc) as tc:
        with (
            tc.tile_pool(name="const", bufs=1) as const,
            tc.tile_pool(name="scal", bufs=1) as sc,
            tc.tile_pool(name="statp", bufs=4) as statp,
            tc.tile_pool(name="sqp", bufs=2) as sqp,
            tc.tile_pool(name="xpool", bufs=2) as xpool,
            tc.tile_pool(name="gpool", bufs=8) as gpool,
            tc.tile_pool(name="spool", bufs=2) as spool,
            tc.tile_pool(name="opool", bufs=4) as opool,
            tc.tile_pool(name="pp", bufs=3, space="PSUM") as pp,
            tc.tile_pool(name="pss", bufs=2, space="PSUM") as pss,
            tc.tile_pool(name="scp", bufs=2, space="PSUM") as scp,
            tc.tile_pool(name="psm", bufs=1, space="PSUM") as psm,
        ):
            # ---------- constants ----------
            w_sb = {}
            for nm in ("y", "v", "p"):
                for ci in range(CCH):
                    t = const.tile([P, C], bf16, tag=f"w{nm}{ci}")
                    nc.sync.dma_start(t[:], wts[nm][ci * P:(ci + 1) * P, :])
                    w_sb[nm, ci] = t
            w1_sb = const.tile([P, CCH], f32, tag="w1col")
            nc.sync.dma_start(w1_sb[:], w1col[:])
            w2_sb = const.tile([P, CCH], f32, tag="w2col")
            nc.sync.dma_start(w2_sb[:], w2col[:])
            pv1_sb = const.tile([P, CCH], f32, tag="pv1col")
            nc.sync.dma_start(pv1_sb[:], pv1col[:])
            pv2_sb = const.tile([P, CCH], f32, tag="pv2col")
            nc.sync.dma_start(pv2_sb[:], pv2col[:])
            mask_sb = const.tile([P, NPR * P], bf16, tag="maskt")
            nc.sync.dma_start(mask_sb[:], maskp[:])
            ocb_sb = const.tile([P, 1], bf16, tag="ocb")
            nc.sync.dma_start(ocb_sb[:], ones_col_b[:])
            orb_sb = const.tile([1, P], bf16, tag="orb")
            nc.sync.dma_start(orb_sb[:], ones_row_b[:])
            omb_sb = const.tile([P, P], bf16, tag="omb")
            nc.sync.dma_start(omb_sb[:], ones_mat_b[:])

            # ---------- stats (sum / sumsq over the whole shard) ----------
            # x-sum via ones-matmuls accumulating in one PSUM bank (PE is
            # idle during the stats phase); sumsq fused into the Square
            # activation's accum_out — DVE does almost no stats work
            nst = nblk * CCH // 2          # 1 MiB stats tiles (2 c-chunks)
            ssq = sc.tile([P, nst], f32, tag="ssq")
            ps_sum1 = psm.tile([1, 512], f32, tag="psm")
            for blk in range(nblk):
                for c2 in range(CCH // 2):
                    xt = statp.tile([P, 2 * WT], bf16, tag="xstat")
                    src = xs[blk, c2 * 2 * P:(c2 + 1) * 2 * P, :].rearrange(
                        "(a p) w -> p a w", p=P)
                    nc.sync.dma_start(xt[:].rearrange("p (a w) -> p a w", a=2),
                                      src)
                    i = blk * (CCH // 2) + c2
                    for j in range(2 * WT // 512):
                        nc.tensor.matmul(
                            ps_sum1[:], ocb_sb[:],
                            xt[:, j * 512:(j + 1) * 512],
                            start=(i == 0 and j == 0),
                            stop=(i == nst - 1 and j == 2 * WT // 512 - 1),
                            skip_group_check=True)
                    sq = sqp.tile([P, 2 * WT], bf16, tag="sq")
                    if i % 3 == 2:
                        # spread the square+reduce work across DVE too so the
                        # stats phase isn't paced by ScalarE alone
                        nc.vector.tensor_mul(sq[:], xt[:], xt[:])
                        nc.vector.reduce_sum(out=ssq[:, i:i + 1], in_=sq[:],
                                             axis=AX)
                    else:
                        nc.scalar.activation(sq[:], xt[:], AF.Square,
                                             accum_out=ssq[:, i:i + 1])
            st_sb = sc.tile([1, 2], f32, tag="st_sb")
            nc.vector.reduce_sum(out=st_sb[0:1, 0:1], in_=ps_sum1[:], axis=AX)
            nc.gpsimd.reduce_sum(out=st_sb[0:1, 1:2], in_=ssq[:],
                                 axis=mybir.AxisListType.XYZWC)
            nc.gpsimd.dma_start(cc_in[:], st_sb[:])
            if use_collective:
                nc.gpsimd.collective_compute(
                    "AllReduce", ALU.add, replica_groups=replica_groups,
                    ins=[cc_in[:]], outs=[cc_out[:]])
            else:
                nc.gpsimd.dma_start(cc_out[:], cc_in[:])
            stg = sc.tile([1, 2], f32, tag="stg")
            nc.gpsimd.dma_start(stg[:], cc_out[:])

            mean = sc.tile([1, 1], f32, tag="mean")
            nc.scalar.activation(mean[:], stg[:, 0:1], AF.Copy,
                                 bias=0.0, scale=1.0 / norm_n)
            ex2 = sc.tile([1, 1], f32, tag="ex2")
            nc.scalar.activation(ex2[:], stg[:, 1:2], AF.Copy,
                                 bias=0.0, scale=1.0 / norm_n)
            msq = sc.tile([1, 1], f32, tag="msq")
            nc.scalar.activation(msq[:], mean[:], AF.Square)
            varp = sc.tile([1, 1], f32, tag="varp")
            nc.vector.tensor_scalar(varp[:], ex2[:], msq[:], EPS,
                                    ALU.subtract, ALU.add)
            sqv = sc.tile([1, 1], f32, tag="sqv")      # = 1/rstd
            nc.scalar.activation(sqv[:], varp[:], AF.Sqrt)
            rst = sc.tile([1, 1], f32, tag="rst")      # = rstd
            nc.vector.reciprocal(rst[:], sqv[:])
            rmu = sc.tile([1, 1], f32, tag="rmu")      # = rstd*mean
            nc.vector.tensor_scalar(rmu[:], mean[:], rst[:], None, ALU.mult)
            rsq = sc.tile([1, 1], f32, tag="rsq")  # = rstd^2
            nc.vector.tensor_scalar(rsq[:], rst[:], rst[:], None, ALU.mult)
            vals = sc.tile([1, 4], f32, tag="vals")
            nc.vector.tensor_copy(vals[:, 0:1], rst[:])
            nc.vector.tensor_copy(vals[:, 1:2], rmu[:])
            nc.vector.tensor_copy(vals[:, 2:3], sqv[:])
            nc.vector.tensor_copy(vals[:, 3:4], rsq[:])
            # broadcast (rstd, rstd*mean, 1/rstd, rstd^2) across partitions
            # on GpSimd — keeps the PE queue free of stats-dependent work
            rb = sc.tile([P, 4], f32, tag="rb")
            nc.gpsimd.partition_broadcast(rb[:], vals[:])
            # all-(1/r) stationary for the softmax denominator matmul: the
            # rowsum matmul then directly yields Z/r, whose reciprocal is the
            # r/Z factor applied at the AV eviction
            oiv = sc.tile([P, P], bf16, tag="oiv")
            nc.vector.tensor_scalar(oiv[:], omb_sb[:], rb[:, 2:3], None,
                                    ALU.mult)
            # score rank-1 vector: v0 = (w1 - rmu*w2)/rstd, so that after the
            # exp's r^2 scale the surviving affine term is r*(Kr^T dq)[s]
            v0c = sc.tile([P, CCH], f32, tag="v0c")
            nc.vector.tensor_scalar(v0c[:], w2_sb[:], rb[:, 1:2], None,
                                    ALU.mult)
            nc.vector.tensor_sub(v0c[:], w1_sb[:], v0c[:])
            v0b = sc.tile([P, CCH], bf16, tag="v0b")
            nc.vector.tensor_scalar(v0b[:], v0c[:], rb[:, 2:3], None,
                                    ALU.mult)
            # dp = Wp @ dv + bp = pv1 - rmu*pv2 (host-folded vectors), the
            # per-channel constant added at P-eviction
            dp = sc.tile([P, CCH], f32, tag="dp")
            nc.vector.tensor_scalar(dp[:], pv2_sb[:], rb[:, 1:2], None,
                                    ALU.mult)
            nc.vector.tensor_sub(dp[:], pv1_sb[:], dp[:])

            # ---------- main blocks ----------
            # One-group software pipeline: stage 1 of group g (projections,
            # scores, exp+mask) is emitted before stage 2 of group g-1
            # (rowsum, AV, P, out), so the softmax latency of g hides under
            # the projection matmuls of g and AV/P of g-1 — the in-order PE
            # queue never waits on ScalarE/DVE.

            def stage1(xb, cs):
                # Y = (Ak^T Aq) x — the single projection that replaces both
                # Q and K: scores are the bilinear form x^T Y
                yg = []
                for co in range(CCH):
                    ps = pp.tile([P, 512], f32, tag="pp")
                    for ci in range(CCH):
                        nc.tensor.matmul(
                            ps[:], w_sb["y", ci][:, co * P:(co + 1) * P],
                            xb[ci][:, cs:cs + 512], start=(ci == 0),
                            stop=(ci == CCH - 1))
                    t = gpool.tile([P, 512], bf16, tag="yg")
                    nc.scalar.copy(t[:], ps[:])
                    yg.append(t)

                # VT (raw): per loc PAIR, [128 (2w,s), 512 co]
                vtp = []
                for p in range(NPR):
                    ps = pss.tile([P, 512], f32, tag="ppv")
                    for ci in range(CCH):
                        nc.tensor.matmul(
                            ps[:], xb[ci][:, cs + p * P:cs + (p + 1) * P],
                            w_sb["v", ci][:], start=(ci == 0),
                            stop=(ci == CCH - 1))
                    t = gpool.tile([P, 512], bf16, tag="vtg")
                    nc.scalar.copy(t[:], ps[:])
                    vtp.append(t)

                # scores S^T[(2w,s), (2w,t)] per pair = x_pair^T Y_pair,
                # plus the rank-1 h[s] x ones_t accumulated on top; 4
                # pair-chains share one PSUM bank: the very first matmul
                # start=True zeroes the bank, later chains' first matmuls
                # overwrite (has_written cleared) and accumulate.
                ps_s = scp.tile([P, 512], f32, tag="pss")
                for p in range(NPR):
                    for ci in range(CCH):
                        nc.tensor.matmul(
                            ps_s[:, p * P:(p + 1) * P],
                            xb[ci][:, cs + p * P:cs + (p + 1) * P],
                            yg[ci][:, p * P:(p + 1) * P],
                            start=(p == 0 and ci == 0),
                            stop=False, skip_group_check=True)
                # score rank-1 bias row h = x^T v0 for this group's columns
                # (cheap: 1-col stationary, N=512 streams); emitted after the
                # S chains so the first groups' projections can run before
                # the stats collective lands
                ps_h = psm.tile([1, 512], f32, tag="psm")
                for ci in range(CCH):
                    nc.tensor.matmul(ps_h[:], v0b[:, ci:ci + 1],
                                     xb[ci][:, cs:cs + 512],
                                     start=(ci == 0), stop=(ci == CCH - 1))
                hrow = spool.tile([1, 512], bf16, tag="hrow")
                with nc.allow_low_precision(
                        reason="bf16 score bias fine at 2e-2 target"):
                    nc.vector.tensor_copy(hrow[:], ps_h[:])
                for p in range(NPR):
                    nc.tensor.matmul(
                        ps_s[:, p * P:(p + 1) * P],
                        hrow[:, p * P:(p + 1) * P], orb_sb[:],
                        start=False, stop=(p == NPR - 1),
                        skip_group_check=True)
                # unnormalized masked softmax numerator; the exp's scale
                # applies the r^2 the bilinear form is missing
                # (normalization is folded into the AV eviction as r/Z)
                pexp = spool.tile([P, 512], bf16, tag="pexp")
                nc.scalar.activation(pexp[:], ps_s[:], AF.Exp,
                                     scale=rb[:, 3:4])
                pm = spool.tile([P, 512], bf16, tag="pmask")
                nc.vector.tensor_mul(pm[:], pexp[:], mask_sb[:])
                return vtp, pm

            def stage2(st):
                xb, cs, blk, vtp, pm = st
                # rowsum matmul with all-(1/r) stationary => Z/r, broadcast
                # across partitions; fast-approx reciprocal gives r/Z with
                # ~18 good bits, plenty for the bf16 og tiles
                ps_sum = psm.tile([P, 512], f32, tag="psm")
                nc.tensor.matmul(ps_sum[:], oiv[:], pm[:],
                                 start=True, stop=True)
                rz = spool.tile([P, 512], f32, tag="rz")
                nc.vector.reciprocal_approx_fast(out=rz[:], in_=ps_sum[:])

                # AV: O[c,(2w,t)] per pair, full-K (mask zeros kill the
                # cross-location contributions); eviction applies r/Z
                og = []
                for ch in range(CCH):
                    ps_o = pp.tile([P, 512], f32, tag="pp")
                    for p in range(NPR):
                        nc.tensor.matmul(
                            ps_o[:, p * P:(p + 1) * P],
                            vtp[p][:, ch * P:(ch + 1) * P],
                            pm[:, p * P:(p + 1) * P],
                            start=(p == 0), stop=True,
                            skip_group_check=True)
                    t = gpool.tile([P, 512], bf16, tag="og")
                    nc.vector.tensor_mul(t[:], ps_o[:], rz[:])
                    og.append(t)

                # P-projection + bias + residual
                for co in range(CCH):
                    ps = pp.tile([P, 512], f32, tag="pp")
                    for ci in range(CCH):
                        nc.tensor.matmul(
                            ps[:], w_sb["p", ci][:, co * P:(co + 1) * P],
                            og[ci][:], start=(ci == 0),
                            stop=(ci == CCH - 1))
                    slab = opool.tile([P, 512], f32, tag="oslab")
                    nc.vector.tensor_scalar(
                        slab[:], ps[:], dp[:, co:co + 1], None, ALU.add)
                    nc.vector.tensor_add(slab[:], slab[:],
                                         xb[co][:, cs:cs + 512])
                    nc.sync.dma_start(
                        outp[blk, co * P:(co + 1) * P, cs:cs + 512],
                        slab[:])

            pending = None
            for blk in range(nblk):
                xb = []
                for ci in range(CCH):
                    t = xpool.tile([P, WT], bf16, tag=f"xb{ci}")
                    nc.sync.dma_start(t[:], xs[blk, ci * P:(ci + 1) * P, :])
                    xb.append(t)
                for g in range(NGRP):
                    cs = g * GRP * T          # 512-col slice of this group
                    vtp, pm = stage1(xb, cs)
                    if pending is not None:
                        stage2(pending)
                    pending = (xb, cs, blk, vtp, pm)
            stage2(pending)
    nc.compile()
    return nc


def host_prep(gamma, beta, wq, bq, wk, bk, wv, bv, wp, bp):
    """Fold gamma/beta into weights; build all constant tensors."""
    s = 1.0 / np.sqrt(np.float64(C))
    g = gamma.astype(np.float64)

    def fold(w, bias, scale):
        a = (w.astype(np.float64) * g[None, :]) * scale      # (co, ci)
        u = (w.astype(np.float64) @ g) * scale               # (co,)
        c0 = (bias.astype(np.float64) + w.astype(np.float64) @
              beta.astype(np.float64)) * scale
        return a, u, c0

    aq, uq, cq = fold(wq, bq, s)
    ak, uk, ck = fold(wk, bk, 1.0)
    av, uv, cv = fold(wv, bv, 1.0)
    # scores are bilinear: S = (Ak x)^T (Aq x) = x^T G x with G = Ak^T Aq;
    # the surviving affine term (s-dependent only — t-terms cancel in
    # softmax) uses w1/w2: h = x^T Ak^T (cq - mu*r*uq)
    G = ak.T @ aq
    w1 = ak.T @ cq
    w2 = ak.T @ uq
    # P-eviction constant dp = Wp@(cv - mu*r*uv) + bp = pv1 - mu*r*pv2
    wp64 = wp.astype(np.float64)
    pv1 = wp64 @ cv + bp.astype(np.float64)
    pv2 = wp64 @ uv
    gyt = np.ascontiguousarray(G.T).astype(BF)
    avt = np.ascontiguousarray(av.T).astype(BF)
    apt = np.ascontiguousarray(wp64.T).astype(BF)

    def colize(v):
        out = np.empty((P, CCH), np.float32)
        for ch in range(CCH):
            out[:, ch] = v[ch * P:(ch + 1) * P]
        return out

    w1c = colize(w1)
    w2c = colize(w2)
    pv1c = colize(pv1)
    pv2c = colize(pv2)

    # pair mask [128, 4*128]: diag 64x64 halves get causal triu (s<=t),
    # off-diag (cross-location) halves are zero; identical per pair.
    tri = np.triu(np.ones((T, T), np.float32))
    blkm = np.zeros((P, P), np.float32)
    blkm[0:T, 0:T] = tri
    blkm[T:2 * T, T:2 * T] = tri
    maskt = np.tile(blkm, (1, NPR))

    consts = {
        "wyt": gyt, "wvt": avt, "wpt": apt,
        "w1col": w1c, "w2col": w2c, "pv1col": pv1c, "pv2col": pv2c,
        "maskt": maskt.astype(BF),
        "ones_col_b": np.ones((P, 1), BF),
        "ones_row_b": np.ones((1, P), BF),
        "ones_mat_b": np.ones((P, P), BF),
    }
    return consts


_NC_CACHE = {}


def kernel(x, gamma, beta, wq, bq, wk, bk, wv, bv, wp, bp):
    x = np.asarray(x, np.float32)
    args = [np.asarray(a, np.float32) for a in
            (gamma, beta, wq, bq, wk, bk, wv, bv, wp, bp)]
    consts = host_prep(*args)

    if "nc" not in _NC_CACHE:
        _NC_CACHE["nc"] = build_nc()
    nc = _NC_CACHE["nc"]

    in_maps = []
    for core in range(NCORES):
        b, hg = core // 4, core % 4
        shard = x[b, :, :, hg * HSH:(hg + 1) * HSH, :]        # (C,T,HSH,W)
        shard = np.ascontiguousarray(
            shard.transpose(2, 0, 3, 1)).reshape(HSH, C, WT)  # w-major
        in_maps.append({"xs": shard.astype(BF), **consts})

    global _last_in_maps
    _last_in_maps = in_maps
    res = run_bass_kernel_spmd(nc, in_maps, list(range(NCORES)))

    out = np.empty((B, C, T, H, W), np.float32)
    for core in range(NCORES):
        b, hg = core // 4, core % 4
        o = res.results[core]["out"].reshape(HSH, C, W, T)
        out[b, :, :, hg * HSH:(hg + 1) * HSH, :] = o.transpose(1, 3, 0, 2)
    return out


# revision 48
# speedup vs baseline: 1.2203x; 1.0197x over previous
"""CausalTemporalAttnBlock Trainium2 kernel.

Problem: out = x + Wp @ attn(norm(x)) + bp, where norm is GroupNorm(1 group)
over (c,t,h,w) per batch, attention is causal over t, independent per (b,h,w).
Shapes: x (2, 512, 64, 32, 32) fp32; four (512,512) weights + biases.

Strategy (8 NeuronCores, zero communication except a 8-byte AllReduce for
the GroupNorm stats):
  - core i handles batch i//4, h-rows [8*(i%4), 8*(i%4)+8), all w: 256 (h,w)
    locations per core.
  - Whole matmul datapath in bf16 (fp32 PSUM accumulation): full-rate PE
    streaming (fp32 is half rate) and fast weight load. Host quantizes x
    and the folded weights to bf16 (RNE); error stays ~1e-3 vs the 2e-2
    budget.
  - Host folds gamma/beta into the projection weights:
        q = r*(Aq @ x) + (cq - mu*r*uq),  Aq = wq*diag(gamma) (q pre-scaled
    by 1/sqrt(c)), uq = wq@gamma, cq = bq + wq@beta; same for k. The V-path
    affine is folded all the way into the P-projection eviction:
        out = x + r*(Wp @ attn @ (Av x)^T) + (Wp @ dv + bp), dv = cv - mu*r*uv
    (softmax rows sum to 1, so the V bias passes through attention as a
    per-channel constant). mu, r=rstd computed on device (AllReduce of
    per-batch sum/sumsq across the 4 cores of each batch).
  - Host re-lays the shard w-major: [8 h-rows][512 c][32 w * 64 t], so one
    attention group (8 w-locations) is a contiguous 512-column slice.
  - Locations are processed in PAIRS sharing the 128-wide stationary
    operand: VT for 2 locations in one matmul (x-pair stationary, Wv
    moving), scores S^T = K^T Q per pair as one [128,128] matmul (cross
    terms masked off with the causal mask), and AV as a full-K [128,128]
    matmul per pair (masked zeros in attn kill the cross contributions).
  - No max-subtraction in softmax (scores are O(1)); causal+pair mask is a
    0/1 multiply after exp; normalization by 1/rowsum via ones-matmul
    reductions/broadcasts on the PE.
"""

import numpy as np
import ml_dtypes

import concourse.bass as bass
import concourse.tile as tile
from concourse import bacc, mybir
from concourse.bass_utils import run_bass_kernel_spmd

P = 128
B, C, T, H, W = 2, 512, 64, 32, 32
NCORES = 8
HSH = H // 4          # 8 h-rows per core
CCH = C // P          # 4 c chunks
GRP = 8               # locations per attention group
NGRP = W // GRP       # 4 groups per h-row block
NPR = GRP // 2        # 4 location-pairs per group
WT = W * T            # 2048 free columns per (h-row, c) plane
EPS = 1e-6

f32 = mybir.dt.float32
bf16 = mybir.dt.bfloat16
AX = mybir.AxisListType.X
ALU = mybir.AluOpType
AF = mybir.ActivationFunctionType
BF = ml_dtypes.bfloat16


def build_nc(num_cores=NCORES, nblk=HSH, norm_n=None, replica_groups=None,
             use_collective=True):
    if norm_n is None:
        norm_n = C * T * H * W
    if replica_groups is None:
        replica_groups = [[0, 1, 2, 3], [4, 5, 6, 7]]
    nc = bacc.Bacc("TRN2", target_bir_lowering=False, debug=False,
                   num_devices=num_cores)

    xs = nc.declare_dram_parameter("xs", [nblk, C, WT], bf16, isOutput=False)
    wts = {}
    for nm in ("y", "v", "p"):
        wts[nm] = nc.declare_dram_parameter(f"w{nm}t", [C, C], bf16,
                                            isOutput=False)
    w1col = nc.declare_dram_parameter("w1col", [P, CCH], f32, isOutput=False)
    w2col = nc.declare_dram_parameter("w2col", [P, CCH], f32, isOutput=False)
    pv1col = nc.declare_dram_parameter("pv1col", [P, CCH], f32, isOutput=False)
    pv2col = nc.declare_dram_parameter("pv2col", [P, CCH], f32, isOutput=False)
    maskp = nc.declare_dram_parameter("maskt", [P, NPR * P], bf16,
                                      isOutput=False)
    ones_col_b = nc.declare_dram_parameter("ones_col_b", [P, 1], bf16,
                                           isOutput=False)
    ones_row_b = nc.declare_dram_parameter("ones_row_b", [1, P], bf16,
                                           isOutput=False)
    ones_mat_b = nc.declare_dram_parameter("ones_mat_b", [P, P], bf16,
                                           isOutput=False)
    outp = nc.declare_dram_parameter("out", [nblk, C, WT], f32, isOutput=True)
    cc_in = nc.dram_tensor("cc_in", [1, 2], f32)
    cc_out = nc.dram_tensor("cc_out", [1, 2], f32)

    with tile.TileContext(nc) as tc:
        with (
            tc.tile_pool(name="const", bufs=1) as const,
            tc.tile_pool(name="scal", bufs=1) as sc,
            tc.tile_pool(name="statp", bufs=4) as statp,
            tc.tile_pool(name="sqp", bufs=2) as sqp,
            tc.tile_pool(name="xpool", bufs=2) as xpool,
            tc.tile_pool(name="gpool", bufs=8) as gpool,
            tc.tile_pool(name="spool", bufs=2) as spool,
            tc.tile_pool(name="opool", bufs=4) as opool,
            tc.tile_pool(name="pp", bufs=3, space="PSUM") as pp,
            tc.tile_pool(name="pss", bufs=2, space="PSUM") as pss,
            tc.tile_pool(name="scp", bufs=2, space="PSUM") as scp,
            tc.tile_pool(name="psm", bufs=1, space="PSUM") as psm,
        ):
            # ---------- constants ----------
            w_sb = {}
            for nm in ("y", "v", "p"):
                for ci in range(CCH):
                    t = const.tile([P, C], bf16, tag=f"w{nm}{ci}")
                    nc.sync.dma_start(t[:], wts[nm][ci * P:(ci + 1) * P, :])
                    w_sb[nm, ci] = t
            w1_sb = const.tile([P, CCH], f32, tag="w1col")
            nc.sync.dma_start(w1_sb[:], w1col[:])
            w2_sb = const.tile([P, CCH], f32, tag="w2col")
            nc.sync.dma_start(w2_sb[:], w2col[:])
            pv1_sb = const.tile([P, CCH], f32, tag="pv1col")
            nc.sync.dma_start(pv1_sb[:], pv1col[:])
            pv2_sb = const.tile([P, CCH], f32, tag="pv2col")
            nc.sync.dma_start(pv2_sb[:], pv2col[:])
            mask_sb = const.tile([P, NPR * P], bf16, tag="maskt")
            nc.sync.dma_start(mask_sb[:], maskp[:])
            ocb_sb = const.tile([P, 1], bf16, tag="ocb")
            nc.sync.dma_start(ocb_sb[:], ones_col_b[:])
            orb_sb = const.tile([1, P], bf16, tag="orb")
            nc.sync.dma_start(orb_sb[:], ones_row_b[:])
            omb_sb = const.tile([P, P], bf16, tag="omb")
            nc.sync.dma_start(omb_sb[:], ones_mat_b[:])

            # ---------- stats (sum / sumsq over the whole shard) ----------
            # x-sum via ones-matmuls accumulating in one PSUM bank (PE is
            # idle during the stats phase); sumsq fused into the Square
            # activation's accum_out — DVE does almost no stats work
            nst = nblk * CCH // 2          # 1 MiB stats tiles (2 c-chunks)
            ssq = sc.tile([P, nst], f32, tag="ssq")
            ps_sum1 = psm.tile([1, 512], f32, tag="psm")
            for blk in range(nblk):
                for c2 in range(CCH // 2):
                    xt = statp.tile([P, 2 * WT], bf16, tag="xstat")
                    src = xs[blk, c2 * 2 * P:(c2 + 1) * 2 * P, :].rearrange(
                        "(a p) w -> p a w", p=P)
                    nc.sync.dma_start(xt[:].rearrange("p (a w) -> p a w", a=2),
                                      src)
                    i = blk * (CCH // 2) + c2
                    for j in range(2 * WT // 512):
                        nc.tensor.matmul(
                            ps_sum1[:], ocb_sb[:],
                            xt[:, j * 512:(j + 1) * 512],
                            start=(i == 0 and j == 0),
                            stop=(i == nst - 1 and j == 2 * WT // 512 - 1),
                            skip_group_check=True)
                    sq = sqp.tile([P, 2 * WT], bf16, tag="sq")
                    if i % 3 == 2:
                        # spread the square+reduce work across DVE too so the
                        # stats phase isn't paced by ScalarE alone
                        nc.vector.tensor_mul(sq[:], xt[:], xt[:])
                        nc.vector.reduce_sum(out=ssq[:, i:i + 1], in_=sq[:],
                                             axis=AX)
                    else:
                        nc.scalar.activation(sq[:], xt[:], AF.Square,
                                             accum_out=ssq[:, i:i + 1])
            st_sb = sc.tile([1, 2], f32, tag="st_sb")
            nc.vector.reduce_sum(out=st_sb[0:1, 0:1], in_=ps_sum1[:], axis=AX)
            nc.gpsimd.reduce_sum(out=st_sb[0:1, 1:2], in_=ssq[:],
                                 axis=mybir.AxisListType.XYZWC)
            nc.gpsimd.dma_start(cc_in[:], st_sb[:])
            if use_collective:
                nc.gpsimd.collective_compute(
                    "AllReduce", ALU.add, replica_groups=replica_groups,
                    ins=[cc_in[:]], outs=[cc_out[:]])
            else:
                nc.gpsimd.dma_start(cc_out[:], cc_in[:])
            stg = sc.tile([1, 2], f32, tag="stg")
            nc.gpsimd.dma_start(stg[:], cc_out[:])

            mean = sc.tile([1, 1], f32, tag="mean")
            nc.scalar.activation(mean[:], stg[:, 0:1], AF.Copy,
                                 bias=0.0, scale=1.0 / norm_n)
            ex2 = sc.tile([1, 1], f32, tag="ex2")
            nc.scalar.activation(ex2[:], stg[:, 1:2], AF.Copy,
                                 bias=0.0, scale=1.0 / norm_n)
            msq = sc.tile([1, 1], f32, tag="msq")
            nc.scalar.activation(msq[:], mean[:], AF.Square)
            varp = sc.tile([1, 1], f32, tag="varp")
            nc.vector.tensor_scalar(varp[:], ex2[:], msq[:], EPS,
                                    ALU.subtract, ALU.add)
            sqv = sc.tile([1, 1], f32, tag="sqv")      # = 1/rstd
            nc.scalar.activation(sqv[:], varp[:], AF.Sqrt)
            rst = sc.tile([1, 1], f32, tag="rst")      # = rstd
            nc.vector.reciprocal(rst[:], sqv[:])
            rmu = sc.tile([1, 1], f32, tag="rmu")      # = rstd*mean
            nc.vector.tensor_scalar(rmu[:], mean[:], rst[:], None, ALU.mult)
            rsq = sc.tile([1, 1], f32, tag="rsq")  # = rstd^2
            nc.vector.tensor_scalar(rsq[:], rst[:], rst[:], None, ALU.mult)
            vals = sc.tile([1, 4], f32, tag="vals")
            nc.vector.tensor_copy(vals[:, 0:1], rst[:])
            nc.vector.tensor_copy(vals[:, 1:2], rmu[:])
            nc.vector.tensor_copy(vals[:, 2:3], sqv[:])
            nc.vector.tensor_copy(vals[:, 3:4], rsq[:])
            # broadcast (rstd, rstd*mean, 1/rstd, rstd^2) across partitions
            # on GpSimd — keeps the PE queue free of stats-dependent work
            rb = sc.tile([P, 4], f32, tag="rb")
            nc.gpsimd.partition_broadcast(rb[:], vals[:])
            # all-(1/r) stationary for the softmax denominator matmul: the
            # rowsum matmul then directly yields Z/r, whose reciprocal is the
            # r/Z factor applied at the AV eviction
            oiv = sc.tile([P, P], bf16, tag="oiv")
            nc.vector.tensor_scalar(oiv[:], omb_sb[:], rb[:, 2:3], None,
                                    ALU.mult)
            # score rank-1 vector: v0 = (w1 - rmu*w2)/rstd, so that after the
            # exp's r^2 scale the surviving affine term is r*(Kr^T dq)[s]
            v0c = sc.tile([P, CCH], f32, tag="v0c")
            nc.vector.tensor_scalar(v0c[:], w2_sb[:], rb[:, 1:2], None,
                                    ALU.mult)
            nc.vector.tensor_sub(v0c[:], w1_sb[:], v0c[:])
            v0b = sc.tile([P, CCH], bf16, tag="v0b")
            nc.vector.tensor_scalar(v0b[:], v0c[:], rb[:, 2:3], None,
                                    ALU.mult)
            # dp = Wp @ dv + bp = pv1 - rmu*pv2 (host-folded vectors), the
            # per-channel constant added at P-eviction
            dp = sc.tile([P, CCH], f32, tag="dp")
            nc.vector.tensor_scalar(dp[:], pv2_sb[:], rb[:, 1:2], None,
                                    ALU.mult)
            nc.vector.tensor_sub(dp[:], pv1_sb[:], dp[:])

            # ---------- main blocks ----------
            # One-group software pipeline: stage 1 of group g (projections,
            # scores, exp+mask) is emitted before stage 2 of group g-1
            # (rowsum, AV, P, out), so the softmax latency of g hides under
            # the projection matmuls of g and AV/P of g-1 — the in-order PE
            # queue never waits on ScalarE/DVE.

            def stage1a(xb, cs):
                # Y = (Ak^T Aq) x — the single projection that replaces both
                # Q and K: scores are the bilinear form x^T Y
                yg = []
                for co in range(CCH):
                    ps = pp.tile([P, 512], f32, tag="pp")
                    for ci in range(CCH):
                        nc.tensor.matmul(
                            ps[:], w_sb["y", ci][:, co * P:(co + 1) * P],
                            xb[ci][:, cs:cs + 512], start=(ci == 0),
                            stop=(ci == CCH - 1))
                    t = gpool.tile([P, 512], bf16, tag="yg")
                    nc.scalar.copy(t[:], ps[:])
                    yg.append(t)

                # VT (raw): per loc PAIR, [128 (2w,s), 512 co]
                vtp = []
                for p in range(NPR):
                    ps = pss.tile([P, 512], f32, tag="ppv")
                    for ci in range(CCH):
                        nc.tensor.matmul(
                            ps[:], xb[ci][:, cs + p * P:cs + (p + 1) * P],
                            w_sb["v", ci][:], start=(ci == 0),
                            stop=(ci == CCH - 1))
                    t = gpool.tile([P, 512], bf16, tag="vtg", bufs=12)
                    nc.scalar.copy(t[:], ps[:])
                    vtp.append(t)

                # scores S^T[(2w,s), (2w,t)] per pair = x_pair^T Y_pair,
                # plus the rank-1 h[s] x ones_t accumulated on top; 4
                # pair-chains share one PSUM bank: the very first matmul
                # start=True zeroes the bank, later chains' first matmuls
                # overwrite (has_written cleared) and accumulate.
                ps_s = scp.tile([P, 512], f32, tag="pss")
                for p in range(NPR):
                    for ci in range(CCH):
                        nc.tensor.matmul(
                            ps_s[:, p * P:(p + 1) * P],
                            xb[ci][:, cs + p * P:cs + (p + 1) * P],
                            yg[ci][:, p * P:(p + 1) * P],
                            start=(p == 0 and ci == 0),
                            stop=False, skip_group_check=True)
                return vtp, ps_s

            def stage1b(st):
                # score rank-1 bias row h = x^T v0 for this group's columns
                # (cheap: 1-col stationary, N=512 streams); lagged one group
                # behind the S chains so the first groups' projections can
                # run before the stats collective lands
                xb, cs, blk, vtp, ps_s = st
                ps_h = psm.tile([1, 512], f32, tag="psm")
                for ci in range(CCH):
                    nc.tensor.matmul(ps_h[:], v0b[:, ci:ci + 1],
                                     xb[ci][:, cs:cs + 512],
                                     start=(ci == 0), stop=(ci == CCH - 1))
                hrow = spool.tile([1, 512], bf16, tag="hrow")
                with nc.allow_low_precision(
                        reason="bf16 score bias fine at 2e-2 target"):
                    nc.vector.tensor_copy(hrow[:], ps_h[:])
                for p in range(NPR):
                    nc.tensor.matmul(
                        ps_s[:, p * P:(p + 1) * P],
                        hrow[:, p * P:(p + 1) * P], orb_sb[:],
                        start=False, stop=(p == NPR - 1),
                        skip_group_check=True)
                # unnormalized masked softmax numerator; the exp's scale
                # applies the r^2 the bilinear form is missing
                # (normalization is folded into the AV eviction as r/Z)
                pexp = spool.tile([P, 512], bf16, tag="pexp")
                nc.scalar.activation(pexp[:], ps_s[:], AF.Exp,
                                     scale=rb[:, 3:4])
                pm = spool.tile([P, 512], bf16, tag="pmask")
                nc.vector.tensor_mul(pm[:], pexp[:], mask_sb[:])
                return xb, cs, blk, vtp, pm

            def stage2(st):
                xb, cs, blk, vtp, pm = st
                # rowsum matmul with all-(1/r) stationary => Z/r, broadcast
                # across partitions; fast-approx reciprocal gives r/Z with
                # ~18 good bits, plenty for the bf16 og tiles
                ps_sum = psm.tile([P, 512], f32, tag="psm")
                nc.tensor.matmul(ps_sum[:], oiv[:], pm[:],
                                 start=True, stop=True)
                rz = spool.tile([P, 512], f32, tag="rz")
                nc.vector.reciprocal_approx_fast(out=rz[:], in_=ps_sum[:])

                # AV: O[c,(2w,t)] per pair, full-K (mask zeros kill the
                # cross-location contributions); eviction applies r/Z
                og = []
                for ch in range(CCH):
                    ps_o = pp.tile([P, 512], f32, tag="pp")
                    for p in range(NPR):
                        nc.tensor.matmul(
                            ps_o[:, p * P:(p + 1) * P],
                            vtp[p][:, ch * P:(ch + 1) * P],
                            pm[:, p * P:(p + 1) * P],
                            start=(p == 0), stop=True,
                            skip_group_check=True)
                    t = gpool.tile([P, 512], bf16, tag="og")
                    nc.vector.tensor_mul(t[:], ps_o[:], rz[:])
                    og.append(t)

                # P-projection + bias + residual
                for co in range(CCH):
                    ps = pp.tile([P, 512], f32, tag="pp")
                    for ci in range(CCH):
                        nc.tensor.matmul(
                            ps[:], w_sb["p", ci][:, co * P:(co + 1) * P],
                            og[ci][:], start=(ci == 0),
                            stop=(ci == CCH - 1))
                    slab = opool.tile([P, 512], f32, tag="oslab")
                    nc.vector.tensor_scalar(
                        slab[:], ps[:], dp[:, co:co + 1], None, ALU.add)
                    nc.vector.tensor_add(slab[:], slab[:],
                                         xb[co][:, cs:cs + 512])
                    nc.sync.dma_start(
                        outp[blk, co * P:(co + 1) * P, cs:cs + 512],
                        slab[:])

            pend1 = None          # awaiting stage1b
            pend2 = None          # awaiting stage2
            for blk in range(nblk):
                xb = []
                for ci in range(CCH):
                    t = xpool.tile([P, WT], bf16, tag=f"xb{ci}")
                    nc.sync.dma_start(t[:], xs[blk, ci * P:(ci + 1) * P, :])
                    xb.append(t)
                for g in range(NGRP):
                    cs = g * GRP * T          # 512-col slice of this group
                    vtp, ps_s = stage1a(xb, cs)
                    nxt2 = stage1b(pend1) if pend1 is not None else None
                    if pend2 is not None:
                        stage2(pend2)
                    pend1 = (xb, cs, blk, vtp, ps_s)
                    pend2 = nxt2
            if pend2 is not None:
                stage2(pend2)
            stage2(stage1b(pend1))
    nc.compile()
    return nc


def host_prep(gamma, beta, wq, bq, wk, bk, wv, bv, wp, bp):
    """Fold gamma/beta into weights; build all constant tensors."""
    s = 1.0 / np.sqrt(np.float64(C))
    g = gamma.astype(np.float64)

    def fold(w, bias, scale):
        a = (w.astype(np.float64) * g[None, :]) * scale      # (co, ci)
        u = (w.astype(np.float64) @ g) * scale               # (co,)
        c0 = (bias.astype(np.float64) + w.astype(np.float64) @
              beta.astype(np.float64)) * scale
        return a, u, c0

    aq, uq, cq = fold(wq, bq, s)
    ak, uk, ck = fold(wk, bk, 1.0)
    av, uv, cv = fold(wv, bv, 1.0)
    # scores are bilinear: S = (Ak x)^T (Aq x) = x^T G x with G = Ak^T Aq;
    # the surviving affine term (s-dependent only — t-terms cancel in
    # softmax) uses w1/w2: h = x^T Ak^T (cq - mu*r*uq)
    G = ak.T @ aq
    w1 = ak.T @ cq
    w2 = ak.T @ uq
    # P-eviction constant dp = Wp@(cv - mu*r*uv) + bp = pv1 - mu*r*pv2
    wp64 = wp.astype(np.float64)
    pv1 = wp64 @ cv + bp.astype(np.float64)
    pv2 = wp64 @ uv
    gyt = np.ascontiguousarray(G.T).astype(BF)
    avt = np.ascontiguousarray(av.T).astype(BF)
    apt = np.ascontiguousarray(wp64.T).astype(BF)

    def colize(v):
        out = np.empty((P, CCH), np.float32)
        for ch in range(CCH):
            out[:, ch] = v[ch * P:(ch + 1) * P]
        return out

    w1c = colize(w1)
    w2c = colize(w2)
    pv1c = colize(pv1)
    pv2c = colize(pv2)

    # pair mask [128, 4*128]: diag 64x64 halves get causal triu (s<=t),
    # off-diag (cross-location) halves are zero; identical per pair.
    tri = np.triu(np.ones((T, T), np.float32))
    blkm = np.zeros((P, P), np.float32)
    blkm[0:T, 0:T] = tri
    blkm[T:2 * T, T:2 * T] = tri
    maskt = np.tile(blkm, (1, NPR))

    consts = {
        "wyt": gyt, "wvt": avt, "wpt": apt,
        "w1col": w1c, "w2col": w2c, "pv1col": pv1c, "pv2col": pv2c,
        "maskt": maskt.astype(BF),
        "ones_col_b": np.ones((P, 1), BF),
        "ones_row_b": np.ones((1, P), BF),
        "ones_mat_b": np.ones((P, P), BF),
    }
    return consts


_NC_CACHE = {}


def kernel(x, gamma, beta, wq, bq, wk, bk, wv, bv, wp, bp):
    x = np.asarray(x, np.float32)
    args = [np.asarray(a, np.float32) for a in
            (gamma, beta, wq, bq, wk, bk, wv, bv, wp, bp)]
    consts = host_prep(*args)

    if "nc" not in _NC_CACHE:
        _NC_CACHE["nc"] = build_nc()
    nc = _NC_CACHE["nc"]

    in_maps = []
    for core in range(NCORES):
        b, hg = core // 4, core % 4
        shard = x[b, :, :, hg * HSH:(hg + 1) * HSH, :]        # (C,T,HSH,W)
        shard = np.ascontiguousarray(
            shard.transpose(2, 0, 3, 1)).reshape(HSH, C, WT)  # w-major
        in_maps.append({"xs": shard.astype(BF), **consts})

    global _last_in_maps
    _last_in_maps = in_maps
    res = run_bass_kernel_spmd(nc, in_maps, list(range(NCORES)))

    out = np.empty((B, C, T, H, W), np.float32)
    for core in range(NCORES):
        b, hg = core // 4, core % 4
        o = res.results[core]["out"].reshape(HSH, C, W, T)
        out[b, :, :, hg * HSH:(hg + 1) * HSH, :] = o.transpose(1, 3, 0, 2)
    return out
